# revision 1
# baseline (speedup 1.0000x reference)
"""Cross-attention with 3D RoPE on 8 Trainium2 NeuronCores.

Sharding: batch*heads across cores. Core i handles batch b=i//4 and heads
(p, p+4) with p=i%4. Per core: q/k/v projections row-sharded over its 2 heads,
attention fully local per head, out-projection column-sharded; the partial
[2048, 768] outputs are summed per batch on the host (sum-gather).

Layout tricks:
- All matmuls run as float32r (fp32 data, ~2e-4 rounding, 4x the fp32 rate).
- Activations X are fed transposed (host-side) so the contraction dim is on
  partitions; q/k are produced directly transposed [d, L] for the S^T matmul.
- q/k head dims are permuted+padded to 128 rows: x1 dims in [0:48), x2 in
  [64:112) (zeros elsewhere, via zero-padded weights). RoPE then needs only
  32-aligned partition slices, which the DVE requires.
- S is computed transposed [k, q]; softmax denominators come for free from a
  ones-column appended to v in the P^T @ v_ones matmul (row 96 of the PV psum).
- No max-subtraction in softmax: |S*scale| stays < ~10, exp is safe in fp32.
"""
import sys

sys.path.insert(0, "/opt/trn_rl_repo")

import numpy as np

B, L, DIM, HEADS, HD = 2, 2048, 768, 8, 96
HDP = 128          # padded head dim for q/k
NC_ = 8            # cores
ROPE_BASE = 10000.0
SCALE = float(HD) ** -0.5
NCHUNK = DIM // 128   # 6 contraction chunks
NLT = L // 512        # 4 free-dim tiles of 512
NKT = L // 128        # 16 k tiles of 128

_nc_cache = {}


def _perm_pad_rows():
    """padded row -> original head-dim index, and the valid-row mask."""
    rows = np.full(HDP, -1, np.int64)
    for r in range(48):
        rows[r] = (r // 16) * 32 + r % 16          # x1 dims
    for r in range(48):
        rows[64 + r] = (r // 16) * 32 + 16 + r % 16  # x2 dims
    return rows


def _freq_mats():
    inv = 1.0 / (ROPE_BASE ** (np.arange(16, dtype=np.float64) / 16.0))
    fc = np.zeros((3, HDP), np.float32)
    fs = np.zeros((3, HDP), np.float32)
    for r in range(48):
        a, j = r // 16, r % 16
        fc[a, r] = inv[j]
        fc[a, 64 + r] = inv[j]
        fs[a, r] = -inv[j]
        fs[a, 64 + r] = inv[j]
    return fc, fs


def _build_program(n_iter=1):
    import concourse.bacc as bacc
    import concourse.mybir as mybir
    from concourse import tile

    F32 = mybir.dt.float32
    F32R = mybir.dt.float32r
    BF16 = mybir.dt.bfloat16
    AF = mybir.ActivationFunctionType

    nc = bacc.Bacc("TRN2", num_devices=NC_)

    # ---- DRAM I/O ----
    xt_q = nc.dram_tensor("xt_q", [DIM, L], F32R, kind="ExternalInput")
    xt_k = nc.dram_tensor("xt_k", [DIM, L], F32R, kind="ExternalInput")
    xt_v = nc.dram_tensor("xt_v", [DIM, L], F32R, kind="ExternalInput")
    wqk = nc.dram_tensor("wqk", [NCHUNK, 128, 4 * HDP], F32R, kind="ExternalInput")
    wv = nc.dram_tensor("wv", [NCHUNK, 128, 256], F32R, kind="ExternalInput")
    wo = nc.dram_tensor("wo", [2, HD, DIM], BF16, kind="ExternalInput")
    # pre-wrapped angle args in [-pi, pi]: cos folded as sin(ang + pi/2)
    ang_in = {
        (tag, kind): nc.dram_tensor(f"a{kind}{tag}", [HDP, L], F32, kind="ExternalInput")
        for tag in ("q", "k")
        for kind in ("c", "sx")
    }
    ones96 = nc.dram_tensor("ones96", [1, HD], F32R, kind="ExternalInput")
    onescol = nc.dram_tensor("onescol", [128, NKT], BF16, kind="ExternalInput")
    out_p = nc.dram_tensor("out_p", [L, DIM], F32, kind="ExternalOutput")

    with tile.TileContext(nc) as tc:
        from contextlib import ExitStack

        ctx = ExitStack()
        with ctx:
            sb_w = ctx.enter_context(tc.tile_pool(name="sb_w", bufs=2))
            sb_cs = ctx.enter_context(tc.tile_pool(name="sb_cs", bufs=1))
            sb_rot = ctx.enter_context(tc.tile_pool(name="sb_rot", bufs=2))
            sb_xt = ctx.enter_context(tc.tile_pool(name="sb_xt", bufs=2))
            sb_v = ctx.enter_context(tc.tile_pool(name="sb_v", bufs=2))
            sb_sc = ctx.enter_context(tc.tile_pool(name="sb_sc", bufs=2))
            sb_ppc = ctx.enter_context(tc.tile_pool(name="sb_ppc", bufs=2))
            sb_pt = ctx.enter_context(tc.tile_pool(name="sb_pt", bufs=6))
            sb_ot = ctx.enter_context(tc.tile_pool(name="sb_ot", bufs=1))
            sb_den = ctx.enter_context(tc.tile_pool(name="sb_den", bufs=1))
            sb_out = ctx.enter_context(tc.tile_pool(name="sb_out", bufs=2))
            ps_big = ctx.enter_context(
                tc.tile_pool(name="ps_big", bufs=1, space="PSUM")
            )
            ps_po = ctx.enter_context(
                tc.tile_pool(name="ps_po", bufs=2, space="PSUM")
            )
            ps_misc = ctx.enter_context(
                tc.tile_pool(name="ps_misc", bufs=2, space="PSUM")
            )

            for _it in range(n_iter):
                # ---- weights ----
                wqk_t = sb_w.tile([128, NCHUNK * 4 * HDP], F32R, name="wqk_t", tag="wqk")
                for wh in range(2):
                    c0, c1 = wh * 3, wh * 3 + 3
                    nc.sync.dma_start(
                        wqk_t[:, c0 * 4 * HDP : c1 * 4 * HDP].rearrange(
                            "p (c f) -> p c f", c=3
                        ),
                        wqk[c0:c1].rearrange("c p f -> p c f"),
                    )
                wv_t = sb_w.tile([128, NCHUNK * 256], F32R, name="wv_t", tag="wv")
                nc.sync.dma_start(
                    wv_t[:].rearrange("p (c f) -> p c f", c=NCHUNK),
                    wv[:].rearrange("c p f -> p c f"),
                )
                ones96_t = sb_w.tile([1, HD], F32R, name="ones96_t", tag="ones")
                nc.sync.dma_start(ones96_t[:], ones96[:])
                wo_t = [
                    sb_w.tile([HD, DIM], BF16, name=f"wo_t{h}", tag=f"wo{h}")
                    for h in range(2)
                ]
                for h2 in range(2):
                    nc.sync.dma_start(wo_t[h2][:], wo[h2])

                # ---- sin/cos of pre-wrapped angles ----
                cs = {}
                for tag in ("q", "k"):
                    c2 = sb_cs.tile([HDP, L], F32, name=f"c2_{tag}", tag=f"c2{tag}")
                    sx = sb_cs.tile([HDP, L], F32, name=f"sx_{tag}", tag=f"sx{tag}")
                    cs[tag] = (c2, sx)

                def emit_sincos():
                    for tag in ("q", "k"):
                        c2, sx = cs[tag]
                        for kind, dst in (("c", c2), ("sx", sx)):
                            nc.gpsimd.dma_start(dst[:], ang_in[(tag, kind)][:])
                            for half in range(2):
                                hs = slice(half * 1024, (half + 1) * 1024)
                                nc.scalar.activation(dst[:, hs], dst[:, hs], AF.Sin)

                rot = {}
                for h in range(2):
                    for tag in ("q", "k"):
                        rot[(tag, h)] = sb_rot.tile(
                            [HDP, L], BF16, name=f"rot_{tag}{h}", tag=f"rot{tag}{h}"
                        )
                v_t = [
                    sb_v.tile([128, NKT * (HD + 1)], BF16, name=f"v_t{h}", tag=f"v{h}")
                    for h in range(2)
                ]
                for h in range(2):
                    nc.sync.dma_start(
                        v_t[h].rearrange("p (k c) -> p k c", c=HD + 1)[:, :, HD],
                        onescol[:],
                    )
                ot = [
                    sb_ot.tile([HD + 1, L], BF16, name=f"ot{h}", tag=f"ot{h}")
                    for h in range(2)
                ]
                otn = [
                    sb_ot.tile([HD, L], BF16, name=f"otn{h}", tag=f"otn{h}")
                    for h in range(2)
                ]
                xt_dram = {"q": xt_q, "k": xt_k}

                proj_pp = {}

                def emit_proj_mm(lt, tags=("k", "q")):
                    sl = slice(lt * 512, (lt + 1) * 512)
                    for tag in tags:
                        base = 0 if tag == "q" else 2 * HDP
                        xtile = sb_xt.tile(
                            [128, NCHUNK * 512], F32R, name=f"xt_{tag}_{lt}", tag="xt"
                        )
                        nc.sync.dma_start(
                            xtile[:].rearrange("p (c l) -> p c l", c=NCHUNK),
                            xt_dram[tag][:, sl].rearrange("(c p) l -> p c l", p=128),
                        )
                        pp = [
                            ps_misc.tile(
                                [HDP, 512], F32, name=f"pp_{tag}{h}_{lt}", tag="m"
                            )
                            for h in range(2)
                        ]
                        for c in range(NCHUNK):
                            for h in range(2):
                                wsl = slice(
                                    c * 4 * HDP + base + h * HDP,
                                    c * 4 * HDP + base + (h + 1) * HDP,
                                )
                                nc.tensor.matmul(
                                    pp[h][:],
                                    wqk_t[:, wsl],
                                    xtile[:, c * 512 : (c + 1) * 512],
                                    start=(c == 0),
                                    stop=(c == NCHUNK - 1),
                                )
                        # drain PSUM fast via scalar (frees ps_misc slots for
                        # the next proj tile while vector chews on rope); the
                        # 64-row half-swap the rope needs comes from an SBUF
                        # DMA since SBUF TTs can't shift partitions
                        ppc = [
                            sb_ppc.tile(
                                [HDP, 512], BF16, name=f"ppc_{tag}{h}_{lt}",
                                tag=f"ppc{h}",
                            )
                            for h in range(2)
                        ]
                        ppcs = [
                            sb_ppc.tile(
                                [HDP, 512], BF16, name=f"ppcs_{tag}{h}_{lt}",
                                tag=f"ppcs{h}",
                            )
                            for h in range(2)
                        ]
                        for h in range(2):
                            nc.scalar.copy(ppc[h][:], pp[h][:])
                            nc.gpsimd.dma_start(ppcs[h][0:64, :], ppc[h][64:128, :])
                            nc.gpsimd.dma_start(ppcs[h][64:128, :], ppc[h][0:64, :])
                        proj_pp[(tag, lt)] = (ppc, ppcs)

                def emit_rope(lt, tags=("k", "q")):
                    sl = slice(lt * 512, (lt + 1) * 512)
                    for tag in tags:
                        c2, sx = cs[tag]
                        ppc, ppcs = proj_pp.pop((tag, lt))
                        for h in range(2):
                            tmp = sb_sc.tile(
                                [HDP, 512], BF16, name=f"tmp_{tag}{h}_{lt}", tag="tmp"
                            )
                            xc = sb_sc.tile(
                                [HDP, 512], BF16, name=f"xc_{tag}{h}_{lt}", tag="xc"
                            )
                            nc.vector.tensor_mul(xc[:], ppc[h][:], c2[:, sl])
                            nc.vector.tensor_mul(tmp[:], ppcs[h][:], sx[:, sl])
                            nc.vector.tensor_add(rot[(tag, h)][:, sl], xc[:], tmp[:])

                def emit_proj_v(ltv):
                    xtv = sb_xt.tile(
                        [128, NCHUNK * 512], F32R, name=f"xtv_{ltv}", tag="xt"
                    )
                    nc.sync.dma_start(
                        xtv[:].rearrange("p (c l) -> p c l", c=NCHUNK),
                        xt_v[:, ltv * 512 : (ltv + 1) * 512].rearrange(
                            "(c p) l -> p c l", p=128
                        ),
                    )
                    for k4 in range(4):
                        kt = ltv * 4 + k4
                        pv = ps_misc.tile([128, 256], F32, name=f"pv_{kt}", tag="m")
                        for c in range(NCHUNK):
                            nc.tensor.matmul(
                                pv[:],
                                xtv[:, c * 512 + k4 * 128 : c * 512 + (k4 + 1) * 128],
                                wv_t[:, c * 256 : (c + 1) * 256],
                                start=(c == 0),
                                stop=(c == NCHUNK - 1),
                            )
                        for h in range(2):
                            nc.vector.tensor_copy(
                                v_t[h][:, kt * (HD + 1) : kt * (HD + 1) + HD],
                                pv[:, h * HD : (h + 1) * HD],
                            )

                pending_norm = []

                def emit_norm(h, qh, recr):
                    for q2 in range(2):
                        qsl = slice((qh * 2 + q2) * 512, (qh * 2 + q2 + 1) * 512)
                        bc = ps_misc.tile(
                            [HD, 512], F32, name=f"bc_{h}_{qh}_{q2}", tag="m"
                        )
                        nc.tensor.matmul(
                            bc[:],
                            ones96_t[:],
                            recr[:, q2 * 512 : (q2 + 1) * 512],
                            start=True,
                            stop=True,
                        )
                        nc.vector.tensor_mul(otn[h][:, qsl], ot[h][0:96, qsl], bc[:])

                attn_po = {}
                attn_pt = {}

                def emit_s_exp(h, qh, kcs):
                    qt_, kt_ = rot[("q", h)], rot[("k", h)]
                    for kc in kcs:
                        ksl = slice(kc * 128, (kc + 1) * 128)
                        st = ps_big.tile(
                            [128, 1024], F32, name=f"st_{h}_{qh}_{kc}", tag="big"
                        )
                        for q2 in range(2):
                            qa = qh * 1024 + q2 * 512
                            nc.tensor.matmul(
                                st[:, q2 * 512 : (q2 + 1) * 512],
                                kt_[:, ksl],
                                qt_[:, qa : qa + 512],
                                start=True,
                                stop=True,
                            )
                        pt = sb_pt.tile(
                            [128, 1024], BF16, name=f"pt_{h}_{qh}_{kc}", tag="pt"
                        )
                        nc.scalar.activation(pt[:], st[:], AF.Exp, scale=SCALE)
                        attn_pt[(h, qh, kc)] = pt

                def emit_pv(h, qh, kcs):
                    if (h, qh) not in attn_po:
                        attn_po[(h, qh)] = ps_po.tile(
                            [HD + 1, 1024], F32, name=f"po_{h}_{qh}", tag="po"
                        )
                    po = attn_po[(h, qh)]
                    for kc in kcs:
                        pt = attn_pt.pop((h, qh, kc))
                        for q2 in range(2):
                            nc.tensor.matmul(
                                po[:, q2 * 512 : (q2 + 1) * 512],
                                v_t[h][:, kc * (HD + 1) : (kc + 1) * (HD + 1)],
                                pt[:, q2 * 512 : (q2 + 1) * 512],
                                start=(kc == 0),
                                stop=(kc == NKT - 1),
                            )
                    if kcs[-1] == NKT - 1:
                        del attn_po[(h, qh)]
                        for q2 in range(2):
                            qa = (qh * 2 + q2) * 512
                            nc.vector.tensor_copy(
                                ot[h][:, qa : qa + 512],
                                po[:, q2 * 512 : (q2 + 1) * 512],
                            )
                        hs = slice(qh * 1024, (qh + 1) * 1024)
                        dr = sb_den.tile([1, 2048], F32, name=f"dr_{h}_{qh}", tag="den")
                        nc.vector.tensor_copy(dr[:, 0:1024], ot[h][96:97, hs])
                        nc.vector.reciprocal_approx_fast(dr[:, 1024:2048], dr[:, 0:1024])
                        recr = sb_den.tile(
                            [1, 1024], F32R, name=f"recr_{h}_{qh}", tag="recr"
                        )
                        nc.vector.tensor_copy(recr[:], dr[:, 1024:2048])
                        pending_norm.append((h, qh, recr))

                # ---- pipelined emission: attention rides the proj DMA; norms and
                # out-projection interleave into later attention streams ----

                def emit_outproj(lt2s):
                    for lt2 in lt2s:
                        lsl = slice(lt2 * 128, (lt2 + 1) * 128)
                        pouts = []
                        for nsl, w in ((slice(0, 512), 512), (slice(512, DIM), 256)):
                            pout = ps_misc.tile(
                                [128, w], F32, name=f"pout_{lt2}_{w}", tag="m"
                            )
                            for h in range(2):
                                nc.tensor.matmul(
                                    pout[:],
                                    otn[h][:, lsl],
                                    wo_t[h][:, nsl],
                                    start=(h == 0),
                                    stop=(h == 1),
                                )
                            pouts.append((pout, nsl))
                        ost = sb_out.tile([128, DIM], F32, name=f"ost_{lt2}", tag="ost")
                        for pout, nsl in pouts:
                            nc.vector.tensor_copy(ost[:, nsl], pout[:])
                        nc.sync.dma_start(out_p[lsl, :], ost[:])

                # Schedule: blocks (0,0) and (1,0) need only q-half 0, so both
                # stream kc-granular through the projection phase (spreading
                # the scalar exp load into the otherwise exp-free proj window);
                # (0,1)/(1,1) fill the rest, with outproj halves pulled as
                # early as their norms allow.
                def emit_group(kcs):
                    for kc in kcs:
                        emit_s_exp(0, 0, [kc])
                        if kc >= 1:
                            emit_pv(0, 0, [kc - 1])
                        emit_s_exp(1, 0, [kc])
                        if kc >= 1:
                            emit_pv(1, 0, [kc - 1])

                emit_sincos()
                for lt in range(NLT):
                    # attention kc-group first (lt>=2) so its exps lead the
                    # scalar queue (ppc copies follow) and the PE has work
                    # while the previous tile's rope drains; lt==1's group
                    # must follow qproj(1) since S reads q cols 0-1023
                    if lt >= 2:
                        emit_group(range((lt - 1) * 4, lt * 4))
                    emit_proj_mm(lt, ("k",))
                    emit_rope(lt, ("k",))
                    emit_proj_mm(lt, ("q",))
                    emit_rope(lt, ("q",))
                    emit_proj_v(lt)
                    if lt == 1:
                        emit_group(range(0, 4))
                emit_group(range(12, NKT))
                emit_s_exp(0, 1, [0])
                emit_pv(0, 0, [15])                          # (0,0) done
                emit_s_exp(0, 1, [1])
                emit_pv(1, 0, [15])                          # (1,0) done
                emit_s_exp(0, 1, [2])
                emit_s_exp(0, 1, [3])
                emit_norm(*pending_norm.pop(0))              # norm(0,0)
                emit_s_exp(0, 1, [4])
                emit_pv(0, 1, [0])
                emit_s_exp(0, 1, [5])
                emit_pv(0, 1, [1])
                emit_norm(*pending_norm.pop(0))              # norm(1,0)
                emit_s_exp(0, 1, [6])
                emit_pv(0, 1, [2])
                emit_s_exp(0, 1, [7])
                emit_pv(0, 1, [3])
                emit_outproj(range(0, 4))
                emit_s_exp(0, 1, [8])
                emit_pv(0, 1, [4])
                emit_s_exp(0, 1, [9])
                emit_pv(0, 1, [5])
                emit_outproj(range(4, 8))
                emit_s_exp(0, 1, [10])
                emit_pv(0, 1, [6])
                emit_s_exp(0, 1, [11])
                emit_pv(0, 1, [7])
                emit_s_exp(1, 1, [0])
                emit_pv(0, 1, [8])
                emit_s_exp(1, 1, [1])
                emit_pv(0, 1, [9])
                emit_s_exp(0, 1, [12])
                emit_pv(0, 1, [10])
                emit_s_exp(0, 1, [13])
                emit_pv(0, 1, [11])
                emit_s_exp(1, 1, [2])
                emit_pv(0, 1, [12])
                emit_s_exp(1, 1, [3])
                emit_pv(0, 1, [13])
                emit_s_exp(0, 1, [14])
                emit_pv(1, 1, [0])
                emit_s_exp(0, 1, [15])
                emit_pv(1, 1, [1])
                emit_s_exp(1, 1, [4])
                emit_pv(0, 1, [14])
                emit_s_exp(1, 1, [5])
                emit_pv(0, 1, [15])                          # (0,1) done
                emit_s_exp(1, 1, [6])
                emit_pv(1, 1, [2])
                emit_s_exp(1, 1, [7])
                emit_pv(1, 1, [3])
                emit_norm(*pending_norm.pop(0))              # norm(0,1)
                emit_s_exp(1, 1, [8])
                emit_pv(1, 1, [4])
                emit_s_exp(1, 1, [9])
                emit_pv(1, 1, [5])
                emit_s_exp(1, 1, [10])
                emit_pv(1, 1, [6])
                emit_s_exp(1, 1, [11])
                emit_pv(1, 1, [7])
                emit_s_exp(1, 1, [12])
                emit_pv(1, 1, [8])
                emit_s_exp(1, 1, [13])
                emit_pv(1, 1, [9])
                emit_s_exp(1, 1, [14])
                emit_pv(1, 1, [10])
                emit_s_exp(1, 1, [15])
                emit_pv(1, 1, [11])
                emit_pv(1, 1, [12, 13])
                emit_pv(1, 1, [14, 15])                      # (1,1) done
                emit_norm(*pending_norm.pop(0))              # norm(1,1)
                emit_outproj(range(8, NKT))

    nc.compile()
    return nc


def _get_program(n_iter=1):
    if n_iter not in _nc_cache:
        _nc_cache[n_iter] = _build_program(n_iter)
    return _nc_cache[n_iter]


def _wrap_pi(x):
    return (x - 2.0 * np.pi * np.round(x / (2.0 * np.pi))).astype(np.float32)


def _angle_tensors(coords):
    """coords [L, 3] -> (AC, ASX) [128, L], wrapped to [-pi, pi]."""
    inv = 1.0 / (ROPE_BASE ** (np.arange(16, dtype=np.float64) / 16.0))
    ang = (coords[:, :, None].astype(np.float64) * inv).transpose(1, 2, 0)  # [3,16,L]
    ang = ang.reshape(48, -1)  # row a*16+j
    ac = np.zeros((HDP, ang.shape[1]), np.float32)
    asx = np.zeros((HDP, ang.shape[1]), np.float32)
    ac[0:48] = _wrap_pi(ang + np.pi / 2)
    ac[64:112] = ac[0:48]
    asx[0:48] = _wrap_pi(-ang)
    asx[64:112] = _wrap_pi(ang)
    return ac, asx


def _host_prep(Q_in, K_in, V_in, coords_q, coords_k, Wq, Wk, Wv, Wo):
    import ml_dtypes

    BF = ml_dtypes.bfloat16
    rows = _perm_pad_rows()
    valid = rows >= 0

    def pad_qk(W, h):
        # [768, 128] transposed, permuted+padded head rows
        Wh = W[h * HD : (h + 1) * HD, :]            # [96, 768]
        P = np.zeros((HDP, DIM), np.float32)
        P[valid] = Wh[rows[valid]]
        return np.ascontiguousarray(P.T)

    xt = {}
    for b in range(B):
        xt[("q", b)] = np.ascontiguousarray(Q_in[b].T)
        xt[("k", b)] = np.ascontiguousarray(K_in[b].T)
        xt[("v", b)] = np.ascontiguousarray(V_in[b].T)
    ang = {}
    for b in range(B):
        ang[("q", b)] = _angle_tensors(coords_q[b])
        ang[("k", b)] = _angle_tensors(coords_k[b])

    ones96 = np.ones((1, HD), np.float32)
    onescol = np.ones((128, NKT), BF)

    in_maps = []
    for core in range(NC_):
        b, p = core // 4, core % 4
        heads = (p, p + 4)
        WQK = np.zeros((NCHUNK, 128, 4 * HDP), np.float32)
        WV = np.zeros((NCHUNK, 128, 256), np.float32)
        WO = np.zeros((2, HD, DIM), BF)
        for hi, h in enumerate(heads):
            wqt = pad_qk(Wq, h)   # [768, 128]
            wkt = pad_qk(Wk, h)
            wvt = np.ascontiguousarray(Wv[h * HD : (h + 1) * HD, :].T)  # [768, 96]
            for c in range(NCHUNK):
                cs_ = slice(c * 128, (c + 1) * 128)
                WQK[c, :, hi * HDP : (hi + 1) * HDP] = wqt[cs_]
                WQK[c, :, 2 * HDP + hi * HDP : 2 * HDP + (hi + 1) * HDP] = wkt[cs_]
                WV[c, :, hi * HD : (hi + 1) * HD] = wvt[cs_]
            WO[hi] = Wo[:, h * HD : (h + 1) * HD].T
        in_maps.append(
            {
                "xt_q": xt[("q", b)],
                "xt_k": xt[("k", b)],
                "xt_v": xt[("v", b)],
                "wqk": WQK,
                "wv": WV,
                "wo": WO,
                "acq": ang[("q", b)][0],
                "asxq": ang[("q", b)][1],
                "ack": ang[("k", b)][0],
                "asxk": ang[("k", b)][1],
                "ones96": ones96,
                "onescol": onescol,
            }
        )
    return in_maps


def kernel(Q_in, K_in, V_in, coords_q, coords_k, Wq, Wk, Wv, Wo, _trace=False):
    from concourse.bass_utils import run_bass_kernel_spmd

    args = [np.asarray(a, np.float32) for a in
            (Q_in, K_in, V_in, coords_q, coords_k, Wq, Wk, Wv, Wo)]
    nc = _get_program()
    in_maps = _host_prep(*args)
    res = run_bass_kernel_spmd(
        nc, in_maps, core_ids=list(range(NC_)), trace=_trace
    )
    out = np.zeros((B, L, DIM), np.float32)
    for core in range(NC_):
        out[core // 4] += res.results[core]["out_p"]
    if _trace:
        kernel._last_results = res
    return out



# revision 12
# speedup vs baseline: 1.2404x; 1.2404x over previous
"""Cross-attention with 3D RoPE on 8 Trainium2 NeuronCores.

Sharding: batch*heads across cores. Core i handles batch b=i//4 and heads
(p, p+4) with p=i%4. Per core: q/k/v projections row-sharded over its 2 heads,
attention fully local per head, out-projection column-sharded; the partial
[2048, 768] outputs are summed per batch on the host (sum-gather).

Layout tricks:
- All matmuls run as float32r (fp32 data, ~2e-4 rounding, 4x the fp32 rate).
- Activations X are fed transposed (host-side) so the contraction dim is on
  partitions; q/k are produced directly transposed [d, L] for the S^T matmul.
- q/k head dims are permuted+padded to 128 rows: x1 dims in [0:48), x2 in
  [64:112) (zeros elsewhere, via zero-padded weights). RoPE then needs only
  32-aligned partition slices, which the DVE requires.
- S is computed transposed [k, q]; softmax denominators come for free from a
  ones-column appended to v in the P^T @ v_ones matmul (row 96 of the PV psum).
- No max-subtraction in softmax: |S*scale| stays < ~10, exp is safe in fp32.
"""
import sys

sys.path.insert(0, "/opt/trn_rl_repo")

import numpy as np

B, L, DIM, HEADS, HD = 2, 2048, 768, 8, 96
HDP = 128          # padded head dim for q/k
NC_ = 8            # cores
ROPE_BASE = 10000.0
SCALE = float(HD) ** -0.5
NCHUNK = DIM // 128   # 6 contraction chunks
NLT = L // 512        # 4 free-dim tiles of 512
NKT = L // 128        # 16 k tiles of 128

_nc_cache = {}


def _perm_pad_rows():
    """padded row -> original head-dim index, and the valid-row mask."""
    rows = np.full(HDP, -1, np.int64)
    for r in range(48):
        rows[r] = (r // 16) * 32 + r % 16          # x1 dims
    for r in range(48):
        rows[64 + r] = (r // 16) * 32 + 16 + r % 16  # x2 dims
    return rows


def _freq_mats():
    inv = 1.0 / (ROPE_BASE ** (np.arange(16, dtype=np.float64) / 16.0))
    fc = np.zeros((3, HDP), np.float32)
    fs = np.zeros((3, HDP), np.float32)
    for r in range(48):
        a, j = r // 16, r % 16
        fc[a, r] = inv[j]
        fc[a, 64 + r] = inv[j]
        fs[a, r] = -inv[j]
        fs[a, 64 + r] = inv[j]
    return fc, fs


def _build_program(n_iter=1):
    import concourse.bacc as bacc
    import concourse.mybir as mybir
    from concourse import tile

    F32 = mybir.dt.float32
    F32R = mybir.dt.float32r
    BF16 = mybir.dt.bfloat16
    AF = mybir.ActivationFunctionType

    nc = bacc.Bacc("TRN2", num_devices=NC_)

    # ---- DRAM I/O ----
    xt_q = nc.dram_tensor("xt_q", [DIM, L], BF16, kind="ExternalInput")
    xt_k = nc.dram_tensor("xt_k", [DIM, L], BF16, kind="ExternalInput")
    xt_v = nc.dram_tensor("xt_v", [DIM, L], BF16, kind="ExternalInput")
    wqk = nc.dram_tensor("wqk", [NCHUNK, 128, 4 * HDP], BF16, kind="ExternalInput")
    wv = nc.dram_tensor("wv", [NCHUNK, 128, 256], BF16, kind="ExternalInput")
    wo = nc.dram_tensor("wo", [2, HD, DIM], BF16, kind="ExternalInput")
    # host-computed cos/sin tables (bf16 values, not angles)
    ang_in = {
        (tag, kind): nc.dram_tensor(f"a{kind}{tag}", [HDP, L], BF16, kind="ExternalInput")
        for tag in ("q", "k")
        for kind in ("c", "sx")
    }
    ones96 = nc.dram_tensor("ones96", [1, HD], F32R, kind="ExternalInput")
    onescol = nc.dram_tensor("onescol", [128, NKT], BF16, kind="ExternalInput")
    out_p = nc.dram_tensor("out_p", [L, DIM], F32, kind="ExternalOutput")

    with tile.TileContext(nc) as tc:
        from contextlib import ExitStack

        ctx = ExitStack()
        with ctx:
            sb_w = ctx.enter_context(tc.tile_pool(name="sb_w", bufs=2))
            sb_cs = ctx.enter_context(tc.tile_pool(name="sb_cs", bufs=1))
            sb_rot = ctx.enter_context(tc.tile_pool(name="sb_rot", bufs=2))
            sb_xt = ctx.enter_context(tc.tile_pool(name="sb_xt", bufs=2))
            sb_v = ctx.enter_context(tc.tile_pool(name="sb_v", bufs=2))
            sb_sc = ctx.enter_context(tc.tile_pool(name="sb_sc", bufs=2))
            sb_ppc = ctx.enter_context(tc.tile_pool(name="sb_ppc", bufs=2))
            sb_pt = ctx.enter_context(tc.tile_pool(name="sb_pt", bufs=6))
            sb_ot = ctx.enter_context(tc.tile_pool(name="sb_ot", bufs=1))
            sb_den = ctx.enter_context(tc.tile_pool(name="sb_den", bufs=1))
            sb_out = ctx.enter_context(tc.tile_pool(name="sb_out", bufs=2))
            ps_big = ctx.enter_context(
                tc.tile_pool(name="ps_big", bufs=1, space="PSUM")
            )
            ps_po = ctx.enter_context(
                tc.tile_pool(name="ps_po", bufs=2, space="PSUM")
            )
            ps_misc = ctx.enter_context(
                tc.tile_pool(name="ps_misc", bufs=2, space="PSUM")
            )

            for _it in range(n_iter):
                # ---- weights ----
                wqk_t = sb_w.tile([128, NCHUNK * 4 * HDP], BF16, name="wqk_t", tag="wqk")
                for wh in range(2):
                    c0, c1 = wh * 3, wh * 3 + 3
                    nc.sync.dma_start(
                        wqk_t[:, c0 * 4 * HDP : c1 * 4 * HDP].rearrange(
                            "p (c f) -> p c f", c=3
                        ),
                        wqk[c0:c1].rearrange("c p f -> p c f"),
                    )
                wv_t = sb_w.tile([128, NCHUNK * 256], BF16, name="wv_t", tag="wv")
                nc.sync.dma_start(
                    wv_t[:].rearrange("p (c f) -> p c f", c=NCHUNK),
                    wv[:].rearrange("c p f -> p c f"),
                )
                ones96_t = sb_w.tile([1, HD], F32R, name="ones96_t", tag="ones")
                nc.sync.dma_start(ones96_t[:], ones96[:])
                wo_t = [
                    sb_w.tile([HD, DIM], BF16, name=f"wo_t{h}", tag=f"wo{h}")
                    for h in range(2)
                ]
                for h2 in range(2):
                    nc.sync.dma_start(wo_t[h2][:], wo[h2])

                # ---- host-computed cos/sin tables, straight DMA ----
                cs = {}
                for tag in ("q", "k"):
                    c2 = sb_cs.tile([HDP, L], BF16, name=f"c2_{tag}", tag=f"c2{tag}")
                    sx = sb_cs.tile([HDP, L], BF16, name=f"sx_{tag}", tag=f"sx{tag}")
                    cs[tag] = (c2, sx)

                def emit_sincos():
                    for tag in ("q", "k"):
                        c2, sx = cs[tag]
                        for kind, dst in (("c", c2), ("sx", sx)):
                            nc.gpsimd.dma_start(dst[:], ang_in[(tag, kind)][:])

                rot = {}
                for h in range(2):
                    for tag in ("q", "k"):
                        rot[(tag, h)] = sb_rot.tile(
                            [HDP, L], BF16, name=f"rot_{tag}{h}", tag=f"rot{tag}{h}"
                        )
                v_t = [
                    sb_v.tile([128, NKT * (HD + 1)], BF16, name=f"v_t{h}", tag=f"v{h}")
                    for h in range(2)
                ]
                for h in range(2):
                    nc.sync.dma_start(
                        v_t[h].rearrange("p (k c) -> p k c", c=HD + 1)[:, :, HD],
                        onescol[:],
                    )
                ot = [
                    sb_ot.tile([HD + 1, L], BF16, name=f"ot{h}", tag=f"ot{h}")
                    for h in range(2)
                ]
                otn = [
                    sb_ot.tile([HD, L], BF16, name=f"otn{h}", tag=f"otn{h}")
                    for h in range(2)
                ]
                xt_dram = {"q": xt_q, "k": xt_k}

                proj_pp = {}

                def emit_proj_mm(lt, tags=("k", "q")):
                    sl = slice(lt * 512, (lt + 1) * 512)
                    for tag in tags:
                        base = 0 if tag == "q" else 2 * HDP
                        xtile = sb_xt.tile(
                            [128, NCHUNK * 512], BF16, name=f"xt_{tag}_{lt}", tag="xt"
                        )
                        nc.sync.dma_start(
                            xtile[:].rearrange("p (c l) -> p c l", c=NCHUNK),
                            xt_dram[tag][:, sl].rearrange("(c p) l -> p c l", p=128),
                        )
                        pp = [
                            ps_misc.tile(
                                [HDP, 512], F32, name=f"pp_{tag}{h}_{lt}", tag="m"
                            )
                            for h in range(2)
                        ]
                        for c in range(NCHUNK):
                            for h in range(2):
                                wsl = slice(
                                    c * 4 * HDP + base + h * HDP,
                                    c * 4 * HDP + base + (h + 1) * HDP,
                                )
                                nc.tensor.matmul(
                                    pp[h][:],
                                    wqk_t[:, wsl],
                                    xtile[:, c * 512 : (c + 1) * 512],
                                    start=(c == 0),
                                    stop=(c == NCHUNK - 1),
                                )
                        # drain PSUM via vector (scalar queue is reserved for
                        # the exp stream); the 64-row half-swap the rope needs
                        # comes from an SBUF DMA since SBUF TTs can't shift
                        # partitions
                        ppc = [
                            sb_ppc.tile(
                                [HDP, 512], BF16, name=f"ppc_{tag}{h}_{lt}",
                                tag=f"ppc{h}",
                            )
                            for h in range(2)
                        ]
                        ppcs = [
                            sb_ppc.tile(
                                [HDP, 512], BF16, name=f"ppcs_{tag}{h}_{lt}",
                                tag=f"ppcs{h}",
                            )
                            for h in range(2)
                        ]
                        for h in range(2):
                            nc.vector.tensor_copy(ppc[h][:], pp[h][:])
                            nc.gpsimd.dma_start(ppcs[h][0:64, :], ppc[h][64:128, :])
                            nc.gpsimd.dma_start(ppcs[h][64:128, :], ppc[h][0:64, :])
                        proj_pp[(tag, lt)] = (ppc, ppcs)

                def emit_rope(lt, tags=("k", "q")):
                    sl = slice(lt * 512, (lt + 1) * 512)
                    for tag in tags:
                        c2, sx = cs[tag]
                        ppc, ppcs = proj_pp.pop((tag, lt))
                        for h in range(2):
                            tmp = sb_sc.tile(
                                [HDP, 512], BF16, name=f"tmp_{tag}{h}_{lt}", tag="tmp"
                            )
                            xc = sb_sc.tile(
                                [HDP, 512], BF16, name=f"xc_{tag}{h}_{lt}", tag="xc"
                            )
                            nc.vector.tensor_mul(xc[:], ppc[h][:], c2[:, sl])
                            nc.vector.tensor_mul(tmp[:], ppcs[h][:], sx[:, sl])
                            nc.vector.tensor_add(rot[(tag, h)][:, sl], xc[:], tmp[:])

                def emit_proj_v(ltv):
                    xtv = sb_xt.tile(
                        [128, NCHUNK * 512], BF16, name=f"xtv_{ltv}", tag="xt"
                    )
                    nc.sync.dma_start(
                        xtv[:].rearrange("p (c l) -> p c l", c=NCHUNK),
                        xt_v[:, ltv * 512 : (ltv + 1) * 512].rearrange(
                            "(c p) l -> p c l", p=128
                        ),
                    )
                    for k4 in range(4):
                        kt = ltv * 4 + k4
                        pv = ps_misc.tile([128, 256], F32, name=f"pv_{kt}", tag="m")
                        for c in range(NCHUNK):
                            nc.tensor.matmul(
                                pv[:],
                                xtv[:, c * 512 + k4 * 128 : c * 512 + (k4 + 1) * 128],
                                wv_t[:, c * 256 : (c + 1) * 256],
                                start=(c == 0),
                                stop=(c == NCHUNK - 1),
                            )
                        for h in range(2):
                            nc.vector.tensor_copy(
                                v_t[h][:, kt * (HD + 1) : kt * (HD + 1) + HD],
                                pv[:, h * HD : (h + 1) * HD],
                            )

                pending_norm = []

                def emit_norm(h, qh, recr):
                    for q2 in range(2):
                        qsl = slice((qh * 2 + q2) * 512, (qh * 2 + q2 + 1) * 512)
                        bc = ps_misc.tile(
                            [HD, 512], F32, name=f"bc_{h}_{qh}_{q2}", tag="m"
                        )
                        nc.tensor.matmul(
                            bc[:],
                            ones96_t[:],
                            recr[:, q2 * 512 : (q2 + 1) * 512],
                            start=True,
                            stop=True,
                        )
                        nc.vector.tensor_mul(otn[h][:, qsl], ot[h][0:96, qsl], bc[:])

                attn_po = {}
                attn_pt = {}

                def emit_s_exp(h, qh, kcs):
                    qt_, kt_ = rot[("q", h)], rot[("k", h)]
                    for kc in kcs:
                        ksl = slice(kc * 128, (kc + 1) * 128)
                        st = ps_big.tile(
                            [128, 1024], F32, name=f"st_{h}_{qh}_{kc}", tag="big"
                        )
                        for q2 in range(2):
                            qa = qh * 1024 + q2 * 512
                            nc.tensor.matmul(
                                st[:, q2 * 512 : (q2 + 1) * 512],
                                kt_[:, ksl],
                                qt_[:, qa : qa + 512],
                                start=True,
                                stop=True,
                            )
                        pt = sb_pt.tile(
                            [128, 1024], BF16, name=f"pt_{h}_{qh}_{kc}", tag="pt"
                        )
                        nc.scalar.activation(pt[:], st[:], AF.Exp, scale=SCALE)
                        attn_pt[(h, qh, kc)] = pt

                def emit_pv(h, qh, kcs):
                    if (h, qh) not in attn_po:
                        attn_po[(h, qh)] = ps_po.tile(
                            [HD + 1, 1024], F32, name=f"po_{h}_{qh}", tag="po"
                        )
                    po = attn_po[(h, qh)]
                    for kc in kcs:
                        pt = attn_pt.pop((h, qh, kc))
                        for q2 in range(2):
                            nc.tensor.matmul(
                                po[:, q2 * 512 : (q2 + 1) * 512],
                                v_t[h][:, kc * (HD + 1) : (kc + 1) * (HD + 1)],
                                pt[:, q2 * 512 : (q2 + 1) * 512],
                                start=(kc == 0),
                                stop=(kc == NKT - 1),
                            )
                    if kcs[-1] == NKT - 1:
                        del attn_po[(h, qh)]
                        for q2 in range(2):
                            qa = (qh * 2 + q2) * 512
                            nc.vector.tensor_copy(
                                ot[h][:, qa : qa + 512],
                                po[:, q2 * 512 : (q2 + 1) * 512],
                            )
                        hs = slice(qh * 1024, (qh + 1) * 1024)
                        dr = sb_den.tile([1, 2048], F32, name=f"dr_{h}_{qh}", tag="den")
                        nc.vector.tensor_copy(dr[:, 0:1024], ot[h][96:97, hs])
                        nc.vector.reciprocal_approx_fast(dr[:, 1024:2048], dr[:, 0:1024])
                        recr = sb_den.tile(
                            [1, 1024], F32R, name=f"recr_{h}_{qh}", tag="recr"
                        )
                        nc.vector.tensor_copy(recr[:], dr[:, 1024:2048])
                        pending_norm.append((h, qh, recr))

                # ---- pipelined emission: attention rides the proj DMA; norms and
                # out-projection interleave into later attention streams ----

                def emit_outproj(lt2s):
                    for lt2 in lt2s:
                        lsl = slice(lt2 * 128, (lt2 + 1) * 128)
                        pouts = []
                        for nsl, w in ((slice(0, 512), 512), (slice(512, DIM), 256)):
                            pout = ps_misc.tile(
                                [128, w], F32, name=f"pout_{lt2}_{w}", tag="m"
                            )
                            for h in range(2):
                                nc.tensor.matmul(
                                    pout[:],
                                    otn[h][:, lsl],
                                    wo_t[h][:, nsl],
                                    start=(h == 0),
                                    stop=(h == 1),
                                )
                            pouts.append((pout, nsl))
                        ost = sb_out.tile([128, DIM], F32, name=f"ost_{lt2}", tag="ost")
                        for pout, nsl in pouts:
                            nc.vector.tensor_copy(ost[:, nsl], pout[:])
                        nc.sync.dma_start(out_p[lsl, :], ost[:])

                # Schedule: blocks (0,0) and (1,0) need only q-half 0, so both
                # stream kc-granular through the projection phase (spreading
                # the scalar exp load into the otherwise exp-free proj window);
                # (0,1)/(1,1) fill the rest, with outproj halves pulled as
                # early as their norms allow.
                def emit_group(kcs):
                    for kc in kcs:
                        emit_s_exp(0, 0, [kc])
                        if kc >= 1:
                            emit_pv(0, 0, [kc - 1])
                        emit_s_exp(1, 0, [kc])
                        if kc >= 1:
                            emit_pv(1, 0, [kc - 1])

                emit_sincos()
                for lt in range(NLT):
                    # attention kc-group first (lt>=2) so its exps lead the
                    # scalar queue (ppc copies follow) and the PE has work
                    # while the previous tile's rope drains; lt==1's group
                    # must follow qproj(1) since S reads q cols 0-1023
                    if lt >= 2:
                        emit_group(range((lt - 1) * 4, lt * 4))
                    emit_proj_mm(lt, ("k",))
                    emit_rope(lt, ("k",))
                    emit_proj_mm(lt, ("q",))
                    emit_rope(lt, ("q",))
                    emit_proj_v(lt)
                    if lt == 1:
                        emit_group(range(0, 4))
                emit_group(range(12, NKT))
                emit_s_exp(0, 1, [0])
                emit_pv(0, 0, [15])                          # (0,0) done
                emit_s_exp(0, 1, [1])
                emit_pv(1, 0, [15])                          # (1,0) done
                emit_s_exp(0, 1, [2])
                emit_s_exp(0, 1, [3])
                emit_norm(*pending_norm.pop(0))              # norm(0,0)
                emit_s_exp(0, 1, [4])
                emit_pv(0, 1, [0])
                emit_s_exp(0, 1, [5])
                emit_pv(0, 1, [1])
                emit_norm(*pending_norm.pop(0))              # norm(1,0)
                emit_s_exp(0, 1, [6])
                emit_pv(0, 1, [2])
                emit_s_exp(0, 1, [7])
                emit_pv(0, 1, [3])
                emit_outproj(range(0, 4))
                emit_s_exp(0, 1, [8])
                emit_pv(0, 1, [4])
                emit_s_exp(0, 1, [9])
                emit_pv(0, 1, [5])
                emit_outproj(range(4, 8))
                emit_s_exp(0, 1, [10])
                emit_pv(0, 1, [6])
                emit_s_exp(0, 1, [11])
                emit_pv(0, 1, [7])
                emit_s_exp(1, 1, [0])
                emit_pv(0, 1, [8])
                emit_s_exp(1, 1, [1])
                emit_pv(0, 1, [9])
                emit_s_exp(0, 1, [12])
                emit_pv(0, 1, [10])
                emit_s_exp(0, 1, [13])
                emit_pv(0, 1, [11])
                emit_s_exp(1, 1, [2])
                emit_pv(0, 1, [12])
                emit_s_exp(1, 1, [3])
                emit_pv(0, 1, [13])
                emit_s_exp(0, 1, [14])
                emit_pv(1, 1, [0])
                emit_s_exp(0, 1, [15])
                emit_pv(1, 1, [1])
                emit_s_exp(1, 1, [4])
                emit_pv(0, 1, [14])
                emit_s_exp(1, 1, [5])
                emit_pv(0, 1, [15])                          # (0,1) done
                emit_s_exp(1, 1, [6])
                emit_pv(1, 1, [2])
                emit_s_exp(1, 1, [7])
                emit_pv(1, 1, [3])
                emit_norm(*pending_norm.pop(0))              # norm(0,1)
                emit_s_exp(1, 1, [8])
                emit_pv(1, 1, [4])
                emit_s_exp(1, 1, [9])
                emit_pv(1, 1, [5])
                emit_s_exp(1, 1, [10])
                emit_pv(1, 1, [6])
                emit_s_exp(1, 1, [11])
                emit_pv(1, 1, [7])
                emit_s_exp(1, 1, [12])
                emit_pv(1, 1, [8])
                emit_s_exp(1, 1, [13])
                emit_pv(1, 1, [9])
                emit_s_exp(1, 1, [14])
                emit_pv(1, 1, [10])
                emit_s_exp(1, 1, [15])
                emit_pv(1, 1, [11])
                emit_pv(1, 1, [12, 13])
                emit_pv(1, 1, [14, 15])                      # (1,1) done
                emit_norm(*pending_norm.pop(0))              # norm(1,1)
                emit_outproj(range(8, NKT))

    nc.compile()
    return nc


def _get_program(n_iter=1):
    if n_iter not in _nc_cache:
        _nc_cache[n_iter] = _build_program(n_iter)
    return _nc_cache[n_iter]


def _angle_tensors(coords):
    """coords [L, 3] -> (COS, SIN') [128, L] bf16 value tables.
    COS rows {0:48, 64:112} = cos(ang); SIN' rows 0:48 = -sin(ang),
    rows 64:112 = +sin(ang); zeros elsewhere keep the padded rot rows zero."""
    import ml_dtypes

    BF = ml_dtypes.bfloat16
    inv = 1.0 / (ROPE_BASE ** (np.arange(16, dtype=np.float64) / 16.0))
    ang = (coords[:, :, None].astype(np.float64) * inv).transpose(1, 2, 0)  # [3,16,L]
    ang = ang.reshape(48, -1)  # row a*16+j
    ac = np.zeros((HDP, ang.shape[1]), BF)
    asx = np.zeros((HDP, ang.shape[1]), BF)
    c = np.cos(ang)
    s = np.sin(ang)
    ac[0:48] = c
    ac[64:112] = c
    asx[0:48] = -s
    asx[64:112] = s
    return ac, asx


def _host_prep(Q_in, K_in, V_in, coords_q, coords_k, Wq, Wk, Wv, Wo):
    import ml_dtypes

    BF = ml_dtypes.bfloat16
    rows = _perm_pad_rows()
    valid = rows >= 0

    def pad_qk(W, h):
        # [768, 128] transposed, permuted+padded head rows
        Wh = W[h * HD : (h + 1) * HD, :]            # [96, 768]
        P = np.zeros((HDP, DIM), np.float32)
        P[valid] = Wh[rows[valid]]
        return np.ascontiguousarray(P.T)

    xt = {}
    for b in range(B):
        xt[("q", b)] = np.ascontiguousarray(Q_in[b].T).astype(BF)
        xt[("k", b)] = np.ascontiguousarray(K_in[b].T).astype(BF)
        xt[("v", b)] = np.ascontiguousarray(V_in[b].T).astype(BF)
    ang = {}
    for b in range(B):
        ang[("q", b)] = _angle_tensors(coords_q[b])
        ang[("k", b)] = _angle_tensors(coords_k[b])

    ones96 = np.ones((1, HD), np.float32)
    onescol = np.ones((128, NKT), BF)

    in_maps = []
    for core in range(NC_):
        b, p = core // 4, core % 4
        heads = (p, p + 4)
        WQK = np.zeros((NCHUNK, 128, 4 * HDP), BF)
        WV = np.zeros((NCHUNK, 128, 256), BF)
        WO = np.zeros((2, HD, DIM), BF)
        for hi, h in enumerate(heads):
            wqt = pad_qk(Wq, h)   # [768, 128]
            wkt = pad_qk(Wk, h)
            wvt = np.ascontiguousarray(Wv[h * HD : (h + 1) * HD, :].T)  # [768, 96]
            for c in range(NCHUNK):
                cs_ = slice(c * 128, (c + 1) * 128)
                WQK[c, :, hi * HDP : (hi + 1) * HDP] = wqt[cs_]
                WQK[c, :, 2 * HDP + hi * HDP : 2 * HDP + (hi + 1) * HDP] = wkt[cs_]
                WV[c, :, hi * HD : (hi + 1) * HD] = wvt[cs_]
            WO[hi] = Wo[:, h * HD : (h + 1) * HD].T
        in_maps.append(
            {
                "xt_q": xt[("q", b)],
                "xt_k": xt[("k", b)],
                "xt_v": xt[("v", b)],
                "wqk": WQK,
                "wv": WV,
                "wo": WO,
                "acq": ang[("q", b)][0],
                "asxq": ang[("q", b)][1],
                "ack": ang[("k", b)][0],
                "asxk": ang[("k", b)][1],
                "ones96": ones96,
                "onescol": onescol,
            }
        )
    return in_maps


def kernel(Q_in, K_in, V_in, coords_q, coords_k, Wq, Wk, Wv, Wo, _trace=False):
    from concourse.bass_utils import run_bass_kernel_spmd

    args = [np.asarray(a, np.float32) for a in
            (Q_in, K_in, V_in, coords_q, coords_k, Wq, Wk, Wv, Wo)]
    nc = _get_program()
    in_maps = _host_prep(*args)
    res = run_bass_kernel_spmd(
        nc, in_maps, core_ids=list(range(NC_)), trace=_trace
    )
    out = np.zeros((B, L, DIM), np.float32)
    for core in range(NC_):
        out[core // 4] += res.results[core]["out_p"]
    if _trace:
        kernel._last_results = res
    return out



# revision 15
# speedup vs baseline: 1.8623x; 1.5013x over previous
"""Cross-attention with 3D RoPE on 8 Trainium2 NeuronCores.

Sharding: batch*heads across cores. Core i handles batch b=i//4 and heads
(p, p+4) with p=i%4. Per core: q/k/v projections row-sharded over its 2 heads,
attention fully local per head, out-projection column-sharded; the partial
[2048, 768] outputs are summed per batch on the host (sum-gather).

Layout:
- All device tensors bf16 except PSUM accumulation and the softmax
  reciprocal chain (f32).
- Activations X are fed transposed (host-side) so the contraction dim is on
  partitions; q/k are produced directly transposed [d, L] for the S^T matmul.
- q/k head dims are permuted+padded to 128 rows: x1 dims in [0:48), x2 in
  [64:112) (zeros elsewhere, via zero-padded weights). RoPE needs only a
  64-row half-swap, done with an SBUF-to-SBUF DMA.
- cos/sin tables are computed on the host and DMA'd in (the scalar engine
  runs nothing but the exp stream, single activation table).
- S is computed transposed [k, q]; softmax denominators come for free from a
  ones-column appended to v in the P^T @ v_ones matmul (row 96 of the PV
  psum). No max-subtraction: |S*scale| < ~10, exp is safe in fp32.
- Normalization: reciprocal on DVE, partition-broadcast on GpSimd, one DVE
  multiply - no PE involvement.

Schedule (per iteration; B0=(h0,q0) B1=(h1,q0) B2=(h0,q1) B3=(h1,q1)):
  prologue: xt DMA prefetch, proj+rope lt0/lt1
  B0 window: attention tiles kc0..15 with proj lt2/lt3 as PE fillers
  B1 window: attention + previous iteration's deferred out-proj (8..15)
  B2 window: attention + norms(B0,B1) + out-proj(0..3)
  B3 window: attention + out-proj(4..7) + NEXT iteration's prologue
  tail: norm(B3), next iteration's lt1 proj; out-proj(8..15) deferred.
The S psum pool is double-buffered so the S stream never serializes with
the scalar exp; PV trails S by 2 tiles.
"""
import sys

sys.path.insert(0, "/opt/trn_rl_repo")

import numpy as np

B, L, DIM, HEADS, HD = 2, 2048, 768, 8, 96
HDP = 128          # padded head dim for q/k
NC_ = 8            # cores
ROPE_BASE = 10000.0
SCALE = float(HD) ** -0.5
NCHUNK = DIM // 128   # 6 contraction chunks
NLT = L // 512        # 4 free-dim tiles of 512
NKT = L // 128        # 16 k tiles of 128
WVW = 2 * HD          # packed v-proj width (192)

_nc_cache = {}


def _perm_pad_rows():
    """padded row -> original head-dim index, and the valid-row mask."""
    rows = np.full(HDP, -1, np.int64)
    for r in range(48):
        rows[r] = (r // 16) * 32 + r % 16          # x1 dims
    for r in range(48):
        rows[64 + r] = (r // 16) * 32 + 16 + r % 16  # x2 dims
    return rows


def _build_program(n_iter=1):
    import concourse.bacc as bacc
    import concourse.mybir as mybir
    from concourse import tile

    F32 = mybir.dt.float32
    BF16 = mybir.dt.bfloat16
    AF = mybir.ActivationFunctionType

    nc = bacc.Bacc("TRN2", num_devices=NC_)

    # ---- DRAM I/O ----
    xt_q = nc.dram_tensor("xt_q", [DIM, L], BF16, kind="ExternalInput")
    xt_k = nc.dram_tensor("xt_k", [DIM, L], BF16, kind="ExternalInput")
    xt_v = nc.dram_tensor("xt_v", [DIM, L], BF16, kind="ExternalInput")
    wqk = nc.dram_tensor("wqk", [NCHUNK, 128, 4 * HDP], BF16, kind="ExternalInput")
    wv = nc.dram_tensor("wv", [NCHUNK, 128, WVW], BF16, kind="ExternalInput")
    wo = nc.dram_tensor("wo", [2, HD, DIM], BF16, kind="ExternalInput")
    # host-computed cos/sin tables (values, not angles)
    ang_in = {
        (tag, kind): nc.dram_tensor(f"a{kind}{tag}", [HDP, L], BF16, kind="ExternalInput")
        for tag in ("q", "k")
        for kind in ("c", "sx")
    }
    onescol = nc.dram_tensor("onescol", [128, NKT], BF16, kind="ExternalInput")
    out_p = nc.dram_tensor("out_p", [L, DIM], F32, kind="ExternalOutput")

    xt_dram = {"q": xt_q, "k": xt_k, "v": xt_v}
    BLOCKS = [(0, 0), (1, 0), (0, 1), (1, 1)]  # (h, qh) windows B0..B3

    with tile.TileContext(nc) as tc:
        from contextlib import ExitStack

        ctx = ExitStack()
        with ctx:
            sb_w = ctx.enter_context(tc.tile_pool(name="sb_w", bufs=2))
            sb_cs = ctx.enter_context(tc.tile_pool(name="sb_cs", bufs=1))
            sb_rot = ctx.enter_context(tc.tile_pool(name="sb_rot", bufs=2))
            sb_xt = ctx.enter_context(tc.tile_pool(name="sb_xt", bufs=5))
            sb_v = ctx.enter_context(tc.tile_pool(name="sb_v", bufs=2))
            sb_sc = ctx.enter_context(tc.tile_pool(name="sb_sc", bufs=2))
            sb_ppc = ctx.enter_context(tc.tile_pool(name="sb_ppc", bufs=2))
            sb_pt = ctx.enter_context(tc.tile_pool(name="sb_pt", bufs=6))
            sb_ot = ctx.enter_context(tc.tile_pool(name="sb_ot", bufs=2))
            sb_den = ctx.enter_context(tc.tile_pool(name="sb_den", bufs=2))
            sb_out = ctx.enter_context(tc.tile_pool(name="sb_out", bufs=2))
            ps_st = ctx.enter_context(
                tc.tile_pool(name="ps_st", bufs=2, space="PSUM")
            )
            ps_po = ctx.enter_context(
                tc.tile_pool(name="ps_po", bufs=1, space="PSUM")
            )
            ps_misc = ctx.enter_context(
                tc.tile_pool(name="ps_misc", bufs=2, space="PSUM")
            )

            def make_ctx(it):
                """Allocate one iteration's tiles, emit its load DMAs, and
                return a dict of emit closures."""
                c = {}

                wqk_t = sb_w.tile(
                    [128, NCHUNK * 4 * HDP], BF16, name=f"wqk_t{it}", tag="wqk"
                )
                for wh in range(2):
                    c0, c1 = wh * 3, wh * 3 + 3
                    nc.sync.dma_start(
                        wqk_t[:, c0 * 4 * HDP : c1 * 4 * HDP].rearrange(
                            "p (c f) -> p c f", c=3
                        ),
                        wqk[c0:c1].rearrange("c p f -> p c f"),
                    )
                wv_t = sb_w.tile([128, NCHUNK * WVW], BF16, name=f"wv_t{it}", tag="wv")
                nc.sync.dma_start(
                    wv_t[:].rearrange("p (c f) -> p c f", c=NCHUNK),
                    wv[:].rearrange("c p f -> p c f"),
                )
                wo_t = [
                    sb_w.tile([HD, DIM], BF16, name=f"wo_t{h}_{it}", tag=f"wo{h}")
                    for h in range(2)
                ]
                for h2 in range(2):
                    nc.sync.dma_start(wo_t[h2][:], wo[h2])

                cs = {}
                for tag in ("q", "k"):
                    c2 = sb_cs.tile([HDP, L], BF16, name=f"c2_{tag}_{it}", tag=f"c2{tag}")
                    sx = sb_cs.tile([HDP, L], BF16, name=f"sx_{tag}_{it}", tag=f"sx{tag}")
                    nc.gpsimd.dma_start(c2[:], ang_in[(tag, "c")][:])
                    nc.gpsimd.dma_start(sx[:], ang_in[(tag, "sx")][:])
                    cs[tag] = (c2, sx)

                rot = {}
                for h in range(2):
                    for tag in ("q", "k"):
                        rot[(tag, h)] = sb_rot.tile(
                            [HDP, L], BF16, name=f"rot_{tag}{h}_{it}", tag=f"rot{tag}{h}"
                        )
                v_t = [
                    sb_v.tile(
                        [128, NKT * (HD + 1)], BF16, name=f"v_t{h}_{it}", tag=f"v{h}"
                    )
                    for h in range(2)
                ]
                for h in range(2):
                    nc.sync.dma_start(
                        v_t[h].rearrange("p (k c) -> p k c", c=HD + 1)[:, :, HD],
                        onescol[:],
                    )
                ot = [
                    sb_ot.tile([HD, L], BF16, name=f"ot{h}_{it}", tag=f"ot{h}")
                    for h in range(2)
                ]
                otn = [
                    sb_ot.tile([HD, L], BF16, name=f"otn{h}_{it}", tag=f"otn{h}")
                    for h in range(2)
                ]

                xt_tiles = {}
                proj_pp = {}
                attn_po = {}
                attn_pt = {}
                rec_of = {}

                def emit_xt_dma(lt, tags=("k", "q", "v")):
                    sl = slice(lt * 512, (lt + 1) * 512)
                    for tag in tags:
                        xtile = sb_xt.tile(
                            [128, NCHUNK * 512], BF16,
                            name=f"xt_{tag}_{lt}_{it}", tag="xt",
                        )
                        nc.sync.dma_start(
                            xtile[:].rearrange("p (c l) -> p c l", c=NCHUNK),
                            xt_dram[tag][:, sl].rearrange("(c p) l -> p c l", p=128),
                        )
                        xt_tiles[(tag, lt)] = xtile

                def emit_proj(lt, tag):
                    base = 0 if tag == "q" else 2 * HDP
                    xtile = xt_tiles.pop((tag, lt))
                    pp = [
                        ps_misc.tile(
                            [HDP, 512], F32, name=f"pp_{tag}{h}_{lt}_{it}", tag="m"
                        )
                        for h in range(2)
                    ]
                    for cc in range(NCHUNK):
                        for h in range(2):
                            wsl = slice(
                                cc * 4 * HDP + base + h * HDP,
                                cc * 4 * HDP + base + (h + 1) * HDP,
                            )
                            nc.tensor.matmul(
                                pp[h][:],
                                wqk_t[:, wsl],
                                xtile[:, cc * 512 : (cc + 1) * 512],
                                start=(cc == 0),
                                stop=(cc == NCHUNK - 1),
                            )
                    # drain PSUM via vector; 64-row half-swap via SBUF DMA
                    ppc = [
                        sb_ppc.tile(
                            [HDP, 512], BF16, name=f"ppc_{tag}{h}_{lt}_{it}",
                            tag=f"ppc{h}",
                        )
                        for h in range(2)
                    ]
                    ppcs = [
                        sb_ppc.tile(
                            [HDP, 512], BF16, name=f"ppcs_{tag}{h}_{lt}_{it}",
                            tag=f"ppcs{h}",
                        )
                        for h in range(2)
                    ]
                    for h in range(2):
                        nc.vector.tensor_copy(ppc[h][:], pp[h][:])
                        nc.gpsimd.dma_start(ppcs[h][0:64, :], ppc[h][64:128, :])
                        nc.gpsimd.dma_start(ppcs[h][64:128, :], ppc[h][0:64, :])
                    proj_pp[(tag, lt)] = (ppc, ppcs)

                def emit_rope(lt, tag):
                    sl = slice(lt * 512, (lt + 1) * 512)
                    c2, sx = cs[tag]
                    ppc, ppcs = proj_pp.pop((tag, lt))
                    for h in range(2):
                        tmp = sb_sc.tile(
                            [HDP, 512], BF16, name=f"tmp_{tag}{h}_{lt}_{it}", tag="tmp"
                        )
                        xc = sb_sc.tile(
                            [HDP, 512], BF16, name=f"xc_{tag}{h}_{lt}_{it}", tag="xc"
                        )
                        nc.vector.tensor_mul(xc[:], ppc[h][:], c2[:, sl])
                        nc.vector.tensor_mul(tmp[:], ppcs[h][:], sx[:, sl])
                        nc.vector.tensor_add(rot[(tag, h)][:, sl], xc[:], tmp[:])

                def emit_proj_v(lt):
                    xtv = xt_tiles.pop(("v", lt))
                    for k4 in range(4):
                        kt = lt * 4 + k4
                        pv = ps_misc.tile(
                            [128, WVW], F32, name=f"pv_{kt}_{it}", tag="m"
                        )
                        for cc in range(NCHUNK):
                            nc.tensor.matmul(
                                pv[:],
                                xtv[:, cc * 512 + k4 * 128 : cc * 512 + (k4 + 1) * 128],
                                wv_t[:, cc * WVW : (cc + 1) * WVW],
                                start=(cc == 0),
                                stop=(cc == NCHUNK - 1),
                            )
                        for h in range(2):
                            nc.vector.tensor_copy(
                                v_t[h][:, kt * (HD + 1) : kt * (HD + 1) + HD],
                                pv[:, h * HD : (h + 1) * HD],
                            )

                def emit_s(h, qh, kc):
                    qt_, kt_ = rot[("q", h)], rot[("k", h)]
                    ksl = slice(kc * 128, (kc + 1) * 128)
                    st = ps_st.tile(
                        [128, 1024], F32, name=f"st_{h}_{qh}_{kc}_{it}", tag="st"
                    )
                    for q2 in range(2):
                        qa = qh * 1024 + q2 * 512
                        nc.tensor.matmul(
                            st[:, q2 * 512 : (q2 + 1) * 512],
                            kt_[:, ksl],
                            qt_[:, qa : qa + 512],
                            start=True,
                            stop=True,
                        )
                    pt = sb_pt.tile(
                        [128, 1024], BF16, name=f"pt_{h}_{qh}_{kc}_{it}", tag="pt"
                    )
                    nc.scalar.activation(pt[:], st[:], AF.Exp, scale=SCALE)
                    attn_pt[(h, qh, kc)] = pt

                def emit_pv(h, qh, kc):
                    if (h, qh) not in attn_po:
                        attn_po[(h, qh)] = ps_po.tile(
                            [HD + 1, 1024], F32, name=f"po_{h}_{qh}_{it}", tag="po"
                        )
                    po = attn_po[(h, qh)]
                    pt = attn_pt.pop((h, qh, kc))
                    for q2 in range(2):
                        nc.tensor.matmul(
                            po[:, q2 * 512 : (q2 + 1) * 512],
                            v_t[h][:, kc * (HD + 1) : (kc + 1) * (HD + 1)],
                            pt[:, q2 * 512 : (q2 + 1) * 512],
                            start=(kc == 0),
                            stop=(kc == NKT - 1),
                        )
                    if kc == NKT - 1:
                        po = attn_po.pop((h, qh))
                        for q2 in range(2):
                            qa = (qh * 2 + q2) * 512
                            nc.vector.tensor_copy(
                                ot[h][:, qa : qa + 512],
                                po[0:HD, q2 * 512 : (q2 + 1) * 512],
                            )
                        dr = sb_den.tile(
                            [1, 1024], F32, name=f"dr_{h}_{qh}_{it}", tag="den"
                        )
                        nc.vector.tensor_copy(dr[:], po[HD : HD + 1, :])
                        rec = sb_den.tile(
                            [1, 1024], F32, name=f"rec_{h}_{qh}_{it}", tag="rec"
                        )
                        nc.vector.reciprocal_approx_fast(rec[:], dr[:])
                        rec_of[(h, qh)] = rec

                def emit_norm(h, qh):
                    hs = slice(qh * 1024, (qh + 1) * 1024)
                    rec = rec_of.pop((h, qh))
                    bc = sb_den.tile(
                        [HD, 1024], F32, name=f"bc_{h}_{qh}_{it}", tag="bc"
                    )
                    nc.gpsimd.partition_broadcast(bc[:], rec[:])
                    nc.vector.tensor_mul(otn[h][:, hs], ot[h][:, hs], bc[:])

                def emit_outproj(lt2):
                    lsl = slice(lt2 * 128, (lt2 + 1) * 128)
                    pouts = []
                    for nsl, w in ((slice(0, 512), 512), (slice(512, DIM), 256)):
                        pout = ps_misc.tile(
                            [128, w], F32, name=f"pout_{lt2}_{w}_{it}", tag="m"
                        )
                        for h in range(2):
                            nc.tensor.matmul(
                                pout[:],
                                otn[h][:, lsl],
                                wo_t[h][:, nsl],
                                start=(h == 0),
                                stop=(h == 1),
                            )
                        pouts.append((pout, nsl))
                    ost = sb_out.tile(
                        [128, DIM], F32, name=f"ost_{lt2}_{it}", tag="ost"
                    )
                    for pout, nsl in pouts:
                        nc.vector.tensor_copy(ost[:, nsl], pout[:])
                    nc.sync.dma_start(out_p[lsl, :], ost[:])

                c.update(
                    xt_dma=emit_xt_dma, proj=emit_proj, rope=emit_rope,
                    proj_v=emit_proj_v, s=emit_s, pv=emit_pv, norm=emit_norm,
                    outproj=emit_outproj,
                )
                return c

            def emit_prologue(c):
                c["xt_dma"](0)
                c["xt_dma"](1, ("k", "q"))
                c["proj"](0, "k")
                c["rope"](0, "k")
                c["proj"](0, "q")
                c["rope"](0, "q")
                c["proj_v"](0)
                c["proj"](1, "k")
                c["rope"](1, "k")
                c["proj"](1, "q")
                c["rope"](1, "q")

            def emit_body(c, make_next, deferred):
                """B windows of iteration c; prologue of the next iteration
                woven into the B3 window. Returns (new_deferred, next_ctx)."""
                # flat attention step list: 64 tiles
                sched = [(h, qh, kc) for (h, qh) in BLOCKS for kc in range(NKT)]

                fillers = {
                    # finish own prologue (lt1 v)
                    0: [lambda: c["xt_dma"](1, ("v",))],
                    1: [lambda: c["proj_v"](1)],
                    2: [lambda: c["xt_dma"](2)],
                    # B0 window: lt2/lt3
                    4: [lambda: c["proj"](2, "k")],
                    5: [lambda: c["rope"](2, "k")],
                    6: [lambda: c["proj"](2, "q")],
                    7: [lambda: c["rope"](2, "q"), lambda: c["xt_dma"](3)],
                    8: [lambda: c["proj_v"](2)],
                    9: [lambda: c["proj"](3, "k")],
                    10: [lambda: c["rope"](3, "k")],
                    11: [lambda: c["proj"](3, "q")],
                    12: [lambda: c["rope"](3, "q"), lambda: c["proj_v"](3)],
                    # B2 window: norms + first outproj half
                    35: [lambda: c["norm"](0, 0)],
                    37: [lambda: c["norm"](1, 0)],
                    38: [lambda: c["outproj"](0)],
                    40: [lambda: c["outproj"](1)],
                    42: [lambda: c["outproj"](2)],
                    44: [lambda: c["outproj"](3)],
                    46: [lambda: c["outproj"](4)],
                    # B3 window: second outproj half
                    49: [lambda: c["outproj"](5)],
                    51: [lambda: c["norm"](0, 1)],
                    52: [lambda: c["outproj"](6)],
                    54: [lambda: c["outproj"](7)],
                }
                # previous iteration's deferred outproj into B1 window
                for i, fn in enumerate(deferred):
                    fillers.setdefault(17 + 2 * i, []).append(fn)

                nxt = None
                LAG = 2
                for t, (h, qh, kc) in enumerate(sched):
                    for fn in fillers.get(t, ()):
                        fn()
                    # next iteration's prologue rides the B3 window
                    if make_next is not None and t == 50:
                        nxt = make_next()
                        emit_prologue(nxt)
                    c["s"](h, qh, kc)
                    if t >= LAG:
                        c["pv"](*sched[t - LAG])
                for t in range(len(sched) - LAG, len(sched)):
                    c["pv"](*sched[t])
                c["norm"](1, 1)
                new_deferred = [
                    (lambda lt2=lt2: c["outproj"](lt2)) for lt2 in range(8, 16)
                ]
                if make_next is None:
                    for fn in new_deferred:
                        fn()
                    new_deferred = []
                return new_deferred, nxt

            cur = make_ctx(0)
            emit_prologue(cur)
            deferred = []
            for n in range(n_iter):
                mk = (lambda it=n + 1: make_ctx(it)) if n + 1 < n_iter else None
                deferred, cur = emit_body(cur, mk, deferred)

    nc.compile()
    return nc


def _get_program(n_iter=1):
    if n_iter not in _nc_cache:
        _nc_cache[n_iter] = _build_program(n_iter)
    return _nc_cache[n_iter]


def _angle_tensors(coords):
    """coords [L, 3] -> (COS, SIN') [128, L] bf16 value tables.
    COS rows {0:48, 64:112} = cos(ang); SIN' rows 0:48 = -sin(ang),
    rows 64:112 = +sin(ang); zeros elsewhere keep the padded rot rows zero."""
    import ml_dtypes

    BF = ml_dtypes.bfloat16
    inv = 1.0 / (ROPE_BASE ** (np.arange(16, dtype=np.float64) / 16.0))
    ang = (coords[:, :, None].astype(np.float64) * inv).transpose(1, 2, 0)  # [3,16,L]
    ang = ang.reshape(48, -1)  # row a*16+j
    ac = np.zeros((HDP, ang.shape[1]), BF)
    asx = np.zeros((HDP, ang.shape[1]), BF)
    cth = np.cos(ang)
    sth = np.sin(ang)
    ac[0:48] = cth
    ac[64:112] = cth
    asx[0:48] = -sth
    asx[64:112] = sth
    return ac, asx


def _host_prep(Q_in, K_in, V_in, coords_q, coords_k, Wq, Wk, Wv, Wo):
    import ml_dtypes

    BF = ml_dtypes.bfloat16
    rows = _perm_pad_rows()
    valid = rows >= 0

    def pad_qk(W, h):
        # [768, 128] transposed, permuted+padded head rows
        Wh = W[h * HD : (h + 1) * HD, :]            # [96, 768]
        P = np.zeros((HDP, DIM), np.float32)
        P[valid] = Wh[rows[valid]]
        return np.ascontiguousarray(P.T)

    xt = {}
    for b in range(B):
        xt[("q", b)] = np.ascontiguousarray(Q_in[b].T).astype(BF)
        xt[("k", b)] = np.ascontiguousarray(K_in[b].T).astype(BF)
        xt[("v", b)] = np.ascontiguousarray(V_in[b].T).astype(BF)
    ang = {}
    for b in range(B):
        ang[("q", b)] = _angle_tensors(coords_q[b])
        ang[("k", b)] = _angle_tensors(coords_k[b])

    onescol = np.ones((128, NKT), BF)

    in_maps = []
    for core in range(NC_):
        b, p = core // 4, core % 4
        heads = (p, p + 4)
        WQK = np.zeros((NCHUNK, 128, 4 * HDP), BF)
        WV = np.zeros((NCHUNK, 128, WVW), BF)
        WO = np.zeros((2, HD, DIM), BF)
        for hi, h in enumerate(heads):
            wqt = pad_qk(Wq, h)   # [768, 128]
            wkt = pad_qk(Wk, h)
            wvt = np.ascontiguousarray(Wv[h * HD : (h + 1) * HD, :].T)  # [768, 96]
            for cc in range(NCHUNK):
                cs_ = slice(cc * 128, (cc + 1) * 128)
                WQK[cc, :, hi * HDP : (hi + 1) * HDP] = wqt[cs_]
                WQK[cc, :, 2 * HDP + hi * HDP : 2 * HDP + (hi + 1) * HDP] = wkt[cs_]
                WV[cc, :, hi * HD : (hi + 1) * HD] = wvt[cs_]
            WO[hi] = Wo[:, h * HD : (h + 1) * HD].T
        in_maps.append(
            {
                "xt_q": xt[("q", b)],
                "xt_k": xt[("k", b)],
                "xt_v": xt[("v", b)],
                "wqk": WQK,
                "wv": WV,
                "wo": WO,
                "acq": ang[("q", b)][0],
                "asxq": ang[("q", b)][1],
                "ack": ang[("k", b)][0],
                "asxk": ang[("k", b)][1],
                "onescol": onescol,
            }
        )
    return in_maps


def kernel(Q_in, K_in, V_in, coords_q, coords_k, Wq, Wk, Wv, Wo, _trace=False):
    from concourse.bass_utils import run_bass_kernel_spmd

    args = [np.asarray(a, np.float32) for a in
            (Q_in, K_in, V_in, coords_q, coords_k, Wq, Wk, Wv, Wo)]
    nc = _get_program()
    in_maps = _host_prep(*args)
    res = run_bass_kernel_spmd(
        nc, in_maps, core_ids=list(range(NC_)), trace=_trace
    )
    out = np.zeros((B, L, DIM), np.float32)
    for core in range(NC_):
        out[core // 4] += res.results[core]["out_p"]
    if _trace:
        kernel._last_results = res
    return out


# revision 16
# speedup vs baseline: 1.9530x; 1.0487x over previous
"""Cross-attention with 3D RoPE on 8 Trainium2 NeuronCores.

Sharding: batch*heads across cores. Core i handles batch b=i//4 and heads
(p, p+4) with p=i%4. Per core: q/k/v projections row-sharded over its 2 heads,
attention fully local per head, out-projection column-sharded; the partial
[2048, 768] outputs are summed per batch on the host (sum-gather).

Layout:
- All device tensors bf16 except PSUM accumulation and the softmax
  reciprocal chain (f32).
- Activations X are fed transposed (host-side) so the contraction dim is on
  partitions; q/k are produced directly transposed [d, L] for the S^T matmul.
- q/k head dims are permuted+padded to 128 rows: x1 dims in [0:48), x2 in
  [64:112) (zeros elsewhere, via zero-padded weights). RoPE needs only a
  64-row half-swap, done with an SBUF-to-SBUF DMA.
- cos/sin tables are computed on the host and DMA'd in (the scalar engine
  runs nothing but the exp stream, single activation table).
- S is computed transposed [k, q]; softmax denominators come for free from a
  ones-column appended to v in the P^T @ v_ones matmul (row 96 of the PV
  psum). No max-subtraction: |S*scale| < ~10, exp is safe in fp32.
- Normalization: reciprocal on DVE, partition-broadcast on GpSimd, one DVE
  multiply - no PE involvement.

Schedule (per iteration; B0=(h0,q0) B1=(h1,q0) B2=(h0,q1) B3=(h1,q1)):
  prologue: xt DMA prefetch, proj+rope lt0/lt1
  B0 window: attention tiles kc0..15 with proj lt2/lt3 as PE fillers
  B1 window: attention + previous iteration's deferred out-proj (8..15)
  B2 window: attention + norms(B0,B1) + out-proj(0..3)
  B3 window: attention + out-proj(4..7) + NEXT iteration's prologue
  tail: norm(B3), next iteration's lt1 proj; out-proj(8..15) deferred.
The S psum pool is double-buffered so the S stream never serializes with
the scalar exp; PV trails S by 2 tiles.
"""
import sys

sys.path.insert(0, "/opt/trn_rl_repo")

import numpy as np

B, L, DIM, HEADS, HD = 2, 2048, 768, 8, 96
HDP = 128          # padded head dim for q/k
NC_ = 8            # cores
ROPE_BASE = 10000.0
SCALE = float(HD) ** -0.5
NCHUNK = DIM // 128   # 6 contraction chunks
NLT = L // 512        # 4 free-dim tiles of 512
NKT = L // 128        # 16 k tiles of 128
WVW = 2 * HD          # packed v-proj width (192)

_nc_cache = {}


def _perm_pad_rows():
    """padded row -> original head-dim index, and the valid-row mask."""
    rows = np.full(HDP, -1, np.int64)
    for r in range(48):
        rows[r] = (r // 16) * 32 + r % 16          # x1 dims
    for r in range(48):
        rows[64 + r] = (r // 16) * 32 + 16 + r % 16  # x2 dims
    return rows


def _build_program(n_iter=1):
    import concourse.bacc as bacc
    import concourse.mybir as mybir
    from concourse import tile

    F32 = mybir.dt.float32
    BF16 = mybir.dt.bfloat16
    AF = mybir.ActivationFunctionType

    nc = bacc.Bacc("TRN2", num_devices=NC_)

    # ---- DRAM I/O ----
    xt_q = nc.dram_tensor("xt_q", [DIM, L], BF16, kind="ExternalInput")
    xt_k = nc.dram_tensor("xt_k", [DIM, L], BF16, kind="ExternalInput")
    xt_v = nc.dram_tensor("xt_v", [DIM, L], BF16, kind="ExternalInput")
    wqk = nc.dram_tensor("wqk", [NCHUNK, 128, 4 * HDP], BF16, kind="ExternalInput")
    wv = nc.dram_tensor("wv", [NCHUNK, 128, WVW], BF16, kind="ExternalInput")
    wo = nc.dram_tensor("wo", [2, HD, DIM], BF16, kind="ExternalInput")
    # host-computed cos/sin tables (values, not angles)
    ang_in = {
        (tag, kind): nc.dram_tensor(f"a{kind}{tag}", [HDP, L], BF16, kind="ExternalInput")
        for tag in ("q", "k")
        for kind in ("c", "sx")
    }
    onescol = nc.dram_tensor("onescol", [128, NKT], BF16, kind="ExternalInput")
    out_p = nc.dram_tensor("out_p", [L, DIM], F32, kind="ExternalOutput")

    xt_dram = {"q": xt_q, "k": xt_k, "v": xt_v}
    BLOCKS = [(0, 0), (1, 0), (0, 1), (1, 1)]  # (h, qh) windows B0..B3

    with tile.TileContext(nc) as tc:
        from contextlib import ExitStack

        ctx = ExitStack()
        with ctx:
            sb_w = ctx.enter_context(tc.tile_pool(name="sb_w", bufs=2))
            sb_cs = ctx.enter_context(tc.tile_pool(name="sb_cs", bufs=1))
            sb_rot = ctx.enter_context(tc.tile_pool(name="sb_rot", bufs=2))
            sb_xt = ctx.enter_context(tc.tile_pool(name="sb_xt", bufs=5))
            sb_v = ctx.enter_context(tc.tile_pool(name="sb_v", bufs=2))
            sb_sc = ctx.enter_context(tc.tile_pool(name="sb_sc", bufs=2))
            sb_ppc = ctx.enter_context(tc.tile_pool(name="sb_ppc", bufs=2))
            sb_pt = ctx.enter_context(tc.tile_pool(name="sb_pt", bufs=6))
            sb_ot = ctx.enter_context(tc.tile_pool(name="sb_ot", bufs=2))
            sb_den = ctx.enter_context(tc.tile_pool(name="sb_den", bufs=2))
            sb_out = ctx.enter_context(tc.tile_pool(name="sb_out", bufs=2))
            ps_st = ctx.enter_context(
                tc.tile_pool(name="ps_st", bufs=2, space="PSUM")
            )
            ps_po = ctx.enter_context(
                tc.tile_pool(name="ps_po", bufs=1, space="PSUM")
            )
            ps_misc = ctx.enter_context(
                tc.tile_pool(name="ps_misc", bufs=2, space="PSUM")
            )

            def make_ctx(it):
                """Allocate one iteration's tiles, emit its load DMAs, and
                return a dict of emit closures."""
                c = {}

                wqk_t = sb_w.tile(
                    [128, NCHUNK * 4 * HDP], BF16, name=f"wqk_t{it}", tag="wqk"
                )
                for wh in range(2):
                    c0, c1 = wh * 3, wh * 3 + 3
                    nc.sync.dma_start(
                        wqk_t[:, c0 * 4 * HDP : c1 * 4 * HDP].rearrange(
                            "p (c f) -> p c f", c=3
                        ),
                        wqk[c0:c1].rearrange("c p f -> p c f"),
                    )
                wv_t = sb_w.tile([128, NCHUNK * WVW], BF16, name=f"wv_t{it}", tag="wv")
                nc.sync.dma_start(
                    wv_t[:].rearrange("p (c f) -> p c f", c=NCHUNK),
                    wv[:].rearrange("c p f -> p c f"),
                )
                wo_t = [
                    sb_w.tile([HD, DIM], BF16, name=f"wo_t{h}_{it}", tag=f"wo{h}")
                    for h in range(2)
                ]
                for h2 in range(2):
                    nc.sync.dma_start(wo_t[h2][:], wo[h2])

                cs = {}
                for tag in ("q", "k"):
                    c2 = sb_cs.tile([HDP, L], BF16, name=f"c2_{tag}_{it}", tag=f"c2{tag}")
                    sx = sb_cs.tile([HDP, L], BF16, name=f"sx_{tag}_{it}", tag=f"sx{tag}")
                    nc.gpsimd.dma_start(c2[:], ang_in[(tag, "c")][:])
                    nc.gpsimd.dma_start(sx[:], ang_in[(tag, "sx")][:])
                    cs[tag] = (c2, sx)

                rot = {}
                for h in range(2):
                    for tag in ("q", "k"):
                        rot[(tag, h)] = sb_rot.tile(
                            [HDP, L], BF16, name=f"rot_{tag}{h}_{it}", tag=f"rot{tag}{h}"
                        )
                v_t = [
                    sb_v.tile(
                        [128, NKT * (HD + 1)], BF16, name=f"v_t{h}_{it}", tag=f"v{h}"
                    )
                    for h in range(2)
                ]
                for h in range(2):
                    nc.sync.dma_start(
                        v_t[h].rearrange("p (k c) -> p k c", c=HD + 1)[:, :, HD],
                        onescol[:],
                    )
                ot = [
                    sb_ot.tile([HD, L], BF16, name=f"ot{h}_{it}", tag=f"ot{h}")
                    for h in range(2)
                ]
                otn = [
                    sb_ot.tile([HD, L], BF16, name=f"otn{h}_{it}", tag=f"otn{h}")
                    for h in range(2)
                ]

                xt_tiles = {}
                proj_pp = {}
                attn_po = {}
                attn_pt = {}
                rec_of = {}

                def emit_xt_dma(lt, tags=("k", "q", "v")):
                    sl = slice(lt * 512, (lt + 1) * 512)
                    for tag in tags:
                        xtile = sb_xt.tile(
                            [128, NCHUNK * 512], BF16,
                            name=f"xt_{tag}_{lt}_{it}", tag="xt",
                        )
                        nc.sync.dma_start(
                            xtile[:].rearrange("p (c l) -> p c l", c=NCHUNK),
                            xt_dram[tag][:, sl].rearrange("(c p) l -> p c l", p=128),
                        )
                        xt_tiles[(tag, lt)] = xtile

                def emit_proj(lt, tag):
                    base = 0 if tag == "q" else 2 * HDP
                    xtile = xt_tiles.pop((tag, lt))
                    pp = [
                        ps_misc.tile(
                            [HDP, 512], F32, name=f"pp_{tag}{h}_{lt}_{it}", tag="m"
                        )
                        for h in range(2)
                    ]
                    for cc in range(NCHUNK):
                        for h in range(2):
                            wsl = slice(
                                cc * 4 * HDP + base + h * HDP,
                                cc * 4 * HDP + base + (h + 1) * HDP,
                            )
                            nc.tensor.matmul(
                                pp[h][:],
                                wqk_t[:, wsl],
                                xtile[:, cc * 512 : (cc + 1) * 512],
                                start=(cc == 0),
                                stop=(cc == NCHUNK - 1),
                            )
                    # drain PSUM via vector; 64-row half-swap via SBUF DMA
                    ppc = [
                        sb_ppc.tile(
                            [HDP, 512], BF16, name=f"ppc_{tag}{h}_{lt}_{it}",
                            tag=f"ppc{h}",
                        )
                        for h in range(2)
                    ]
                    ppcs = [
                        sb_ppc.tile(
                            [HDP, 512], BF16, name=f"ppcs_{tag}{h}_{lt}_{it}",
                            tag=f"ppcs{h}",
                        )
                        for h in range(2)
                    ]
                    for h in range(2):
                        nc.vector.tensor_copy(ppc[h][:], pp[h][:])
                        nc.gpsimd.dma_start(ppcs[h][0:64, :], ppc[h][64:128, :])
                        nc.gpsimd.dma_start(ppcs[h][64:128, :], ppc[h][0:64, :])
                    proj_pp[(tag, lt)] = (ppc, ppcs)

                def emit_rope(lt, tag):
                    sl = slice(lt * 512, (lt + 1) * 512)
                    c2, sx = cs[tag]
                    ppc, ppcs = proj_pp.pop((tag, lt))
                    for h in range(2):
                        tmp = sb_sc.tile(
                            [HDP, 512], BF16, name=f"tmp_{tag}{h}_{lt}_{it}", tag="tmp"
                        )
                        xc = sb_sc.tile(
                            [HDP, 512], BF16, name=f"xc_{tag}{h}_{lt}_{it}", tag="xc"
                        )
                        nc.vector.tensor_mul(xc[:], ppc[h][:], c2[:, sl])
                        nc.vector.tensor_mul(tmp[:], ppcs[h][:], sx[:, sl])
                        nc.vector.tensor_add(rot[(tag, h)][:, sl], xc[:], tmp[:])

                def emit_proj_v(lt):
                    xtv = xt_tiles.pop(("v", lt))
                    for k4 in range(4):
                        kt = lt * 4 + k4
                        pv = ps_misc.tile(
                            [128, WVW], F32, name=f"pv_{kt}_{it}", tag="m"
                        )
                        for cc in range(NCHUNK):
                            nc.tensor.matmul(
                                pv[:],
                                xtv[:, cc * 512 + k4 * 128 : cc * 512 + (k4 + 1) * 128],
                                wv_t[:, cc * WVW : (cc + 1) * WVW],
                                start=(cc == 0),
                                stop=(cc == NCHUNK - 1),
                            )
                        for h in range(2):
                            nc.vector.tensor_copy(
                                v_t[h][:, kt * (HD + 1) : kt * (HD + 1) + HD],
                                pv[:, h * HD : (h + 1) * HD],
                            )

                def emit_s(h, qh, kc):
                    qt_, kt_ = rot[("q", h)], rot[("k", h)]
                    ksl = slice(kc * 128, (kc + 1) * 128)
                    st = ps_st.tile(
                        [128, 1024], F32, name=f"st_{h}_{qh}_{kc}_{it}", tag="st"
                    )
                    for q2 in range(2):
                        qa = qh * 1024 + q2 * 512
                        nc.tensor.matmul(
                            st[:, q2 * 512 : (q2 + 1) * 512],
                            kt_[:, ksl],
                            qt_[:, qa : qa + 512],
                            start=True,
                            stop=True,
                        )
                    pt = sb_pt.tile(
                        [128, 1024], BF16, name=f"pt_{h}_{qh}_{kc}_{it}", tag="pt"
                    )
                    nc.scalar.activation(pt[:], st[:], AF.Exp, scale=SCALE)
                    attn_pt[(h, qh, kc)] = pt

                def emit_pv(h, qh, kc):
                    if (h, qh) not in attn_po:
                        attn_po[(h, qh)] = ps_po.tile(
                            [HD + 1, 1024], F32, name=f"po_{h}_{qh}_{it}", tag="po"
                        )
                    po = attn_po[(h, qh)]
                    pt = attn_pt.pop((h, qh, kc))
                    for q2 in range(2):
                        nc.tensor.matmul(
                            po[:, q2 * 512 : (q2 + 1) * 512],
                            v_t[h][:, kc * (HD + 1) : (kc + 1) * (HD + 1)],
                            pt[:, q2 * 512 : (q2 + 1) * 512],
                            start=(kc == 0),
                            stop=(kc == NKT - 1),
                        )
                    if kc == NKT - 1:
                        po = attn_po.pop((h, qh))
                        for q2 in range(2):
                            qa = (qh * 2 + q2) * 512
                            nc.vector.tensor_copy(
                                ot[h][:, qa : qa + 512],
                                po[0:HD, q2 * 512 : (q2 + 1) * 512],
                            )
                        dr = sb_den.tile(
                            [1, 1024], F32, name=f"dr_{h}_{qh}_{it}", tag="den"
                        )
                        nc.vector.tensor_copy(dr[:], po[HD : HD + 1, :])
                        rec = sb_den.tile(
                            [1, 1024], F32, name=f"rec_{h}_{qh}_{it}", tag="rec"
                        )
                        nc.vector.reciprocal_approx_fast(rec[:], dr[:])
                        rec_of[(h, qh)] = rec

                def emit_norm(h, qh):
                    hs = slice(qh * 1024, (qh + 1) * 1024)
                    rec = rec_of.pop((h, qh))
                    bc = sb_den.tile(
                        [HD, 1024], F32, name=f"bc_{h}_{qh}_{it}", tag="bc"
                    )
                    nc.gpsimd.partition_broadcast(bc[:], rec[:])
                    nc.vector.tensor_mul(otn[h][:, hs], ot[h][:, hs], bc[:])

                def emit_outproj(lt2):
                    lsl = slice(lt2 * 128, (lt2 + 1) * 128)
                    pouts = []
                    for nsl, w in ((slice(0, 512), 512), (slice(512, DIM), 256)):
                        pout = ps_misc.tile(
                            [128, w], F32, name=f"pout_{lt2}_{w}_{it}", tag="m"
                        )
                        for h in range(2):
                            nc.tensor.matmul(
                                pout[:],
                                otn[h][:, lsl],
                                wo_t[h][:, nsl],
                                start=(h == 0),
                                stop=(h == 1),
                            )
                        pouts.append((pout, nsl))
                    ost = sb_out.tile(
                        [128, DIM], F32, name=f"ost_{lt2}_{it}", tag="ost"
                    )
                    for pout, nsl in pouts:
                        nc.vector.tensor_copy(ost[:, nsl], pout[:])
                    nc.sync.dma_start(out_p[lsl, :], ost[:])

                c.update(
                    xt_dma=emit_xt_dma, proj=emit_proj, rope=emit_rope,
                    proj_v=emit_proj_v, s=emit_s, pv=emit_pv, norm=emit_norm,
                    outproj=emit_outproj,
                )
                return c

            def emit_prologue_dma(c):
                c["xt_dma"](0)
                c["xt_dma"](1, ("k", "q"))

            def prologue_units(c):
                return [
                    lambda: c["proj"](0, "k"),
                    lambda: (c["rope"](0, "k"), c["xt_dma"](1, ("v",))),
                    lambda: c["proj"](0, "q"),
                    lambda: c["rope"](0, "q"),
                    lambda: c["proj_v"](0),
                    lambda: c["proj"](1, "k"),
                    lambda: c["rope"](1, "k"),
                    lambda: c["proj"](1, "q"),
                    lambda: c["rope"](1, "q"),
                    lambda: c["proj_v"](1),
                ]

            def emit_body(c, make_next, deferred):
                """B windows of iteration c; prologue of the next iteration
                woven into the B3 window. Returns (new_deferred, next_ctx)."""
                # flat attention step list: 64 tiles
                sched = [(h, qh, kc) for (h, qh) in BLOCKS for kc in range(NKT)]

                fillers = {
                    2: [lambda: c["xt_dma"](2)],
                    # B0 window: lt2/lt3
                    4: [lambda: c["proj"](2, "k")],
                    5: [lambda: c["rope"](2, "k")],
                    6: [lambda: c["proj"](2, "q")],
                    7: [lambda: c["rope"](2, "q"), lambda: c["xt_dma"](3)],
                    8: [lambda: c["proj_v"](2)],
                    9: [lambda: c["proj"](3, "k")],
                    10: [lambda: c["rope"](3, "k")],
                    11: [lambda: c["proj"](3, "q")],
                    12: [lambda: c["rope"](3, "q"), lambda: c["proj_v"](3)],
                    # norms as soon as their block drains
                    20: [lambda: c["norm"](0, 0)],
                    35: [lambda: c["norm"](1, 0)],
                    # outproj spread over B2/B3 windows
                    37: [lambda: c["outproj"](0)],
                    39: [lambda: c["outproj"](1)],
                    41: [lambda: c["outproj"](2)],
                    43: [lambda: c["outproj"](3)],
                    45: [lambda: c["outproj"](4)],
                    48: [lambda: c["outproj"](5)],
                    50: [lambda: c["outproj"](6)],
                    51: [lambda: c["norm"](0, 1)],
                    53: [lambda: c["outproj"](7)],
                }
                # previous iteration's deferred outproj into B1 window
                for i, fn in enumerate(deferred):
                    fillers.setdefault(17 + 2 * i, []).append(fn)

                nxt = None
                LAG = 2
                for t, (h, qh, kc) in enumerate(sched):
                    # next iteration's tiles + DMAs early in B3 window, its
                    # prologue compute spread over the window's steps
                    if make_next is not None and t == 46:
                        nxt = make_next()
                        emit_prologue_dma(nxt)
                        for i, fn in enumerate(prologue_units(nxt)):
                            fillers.setdefault(49 + (3 * i) // 2, []).append(fn)
                    for fn in fillers.get(t, ()):
                        fn()
                    c["s"](h, qh, kc)
                    if t >= LAG:
                        c["pv"](*sched[t - LAG])
                for t in range(len(sched) - LAG, len(sched)):
                    c["pv"](*sched[t])
                c["norm"](1, 1)
                new_deferred = [
                    (lambda lt2=lt2: c["outproj"](lt2)) for lt2 in range(8, 16)
                ]
                if make_next is None:
                    for fn in new_deferred:
                        fn()
                    new_deferred = []
                return new_deferred, nxt

            cur = make_ctx(0)
            emit_prologue_dma(cur)
            for fn in prologue_units(cur):
                fn()
            deferred = []
            for n in range(n_iter):
                mk = (lambda it=n + 1: make_ctx(it)) if n + 1 < n_iter else None
                deferred, cur = emit_body(cur, mk, deferred)

    nc.compile()
    return nc


def _get_program(n_iter=1):
    if n_iter not in _nc_cache:
        _nc_cache[n_iter] = _build_program(n_iter)
    return _nc_cache[n_iter]


def _angle_tensors(coords):
    """coords [L, 3] -> (COS, SIN') [128, L] bf16 value tables.
    COS rows {0:48, 64:112} = cos(ang); SIN' rows 0:48 = -sin(ang),
    rows 64:112 = +sin(ang); zeros elsewhere keep the padded rot rows zero."""
    import ml_dtypes

    BF = ml_dtypes.bfloat16
    inv = 1.0 / (ROPE_BASE ** (np.arange(16, dtype=np.float64) / 16.0))
    ang = (coords[:, :, None].astype(np.float64) * inv).transpose(1, 2, 0)  # [3,16,L]
    ang = ang.reshape(48, -1)  # row a*16+j
    ac = np.zeros((HDP, ang.shape[1]), BF)
    asx = np.zeros((HDP, ang.shape[1]), BF)
    cth = np.cos(ang)
    sth = np.sin(ang)
    ac[0:48] = cth
    ac[64:112] = cth
    asx[0:48] = -sth
    asx[64:112] = sth
    return ac, asx


def _host_prep(Q_in, K_in, V_in, coords_q, coords_k, Wq, Wk, Wv, Wo):
    import ml_dtypes

    BF = ml_dtypes.bfloat16
    rows = _perm_pad_rows()
    valid = rows >= 0

    def pad_qk(W, h):
        # [768, 128] transposed, permuted+padded head rows
        Wh = W[h * HD : (h + 1) * HD, :]            # [96, 768]
        P = np.zeros((HDP, DIM), np.float32)
        P[valid] = Wh[rows[valid]]
        return np.ascontiguousarray(P.T)

    xt = {}
    for b in range(B):
        xt[("q", b)] = np.ascontiguousarray(Q_in[b].T).astype(BF)
        xt[("k", b)] = np.ascontiguousarray(K_in[b].T).astype(BF)
        xt[("v", b)] = np.ascontiguousarray(V_in[b].T).astype(BF)
    ang = {}
    for b in range(B):
        ang[("q", b)] = _angle_tensors(coords_q[b])
        ang[("k", b)] = _angle_tensors(coords_k[b])

    onescol = np.ones((128, NKT), BF)

    in_maps = []
    for core in range(NC_):
        b, p = core // 4, core % 4
        heads = (p, p + 4)
        WQK = np.zeros((NCHUNK, 128, 4 * HDP), BF)
        WV = np.zeros((NCHUNK, 128, WVW), BF)
        WO = np.zeros((2, HD, DIM), BF)
        for hi, h in enumerate(heads):
            wqt = pad_qk(Wq, h)   # [768, 128]
            wkt = pad_qk(Wk, h)
            wvt = np.ascontiguousarray(Wv[h * HD : (h + 1) * HD, :].T)  # [768, 96]
            for cc in range(NCHUNK):
                cs_ = slice(cc * 128, (cc + 1) * 128)
                WQK[cc, :, hi * HDP : (hi + 1) * HDP] = wqt[cs_]
                WQK[cc, :, 2 * HDP + hi * HDP : 2 * HDP + (hi + 1) * HDP] = wkt[cs_]
                WV[cc, :, hi * HD : (hi + 1) * HD] = wvt[cs_]
            WO[hi] = Wo[:, h * HD : (h + 1) * HD].T
        in_maps.append(
            {
                "xt_q": xt[("q", b)],
                "xt_k": xt[("k", b)],
                "xt_v": xt[("v", b)],
                "wqk": WQK,
                "wv": WV,
                "wo": WO,
                "acq": ang[("q", b)][0],
                "asxq": ang[("q", b)][1],
                "ack": ang[("k", b)][0],
                "asxk": ang[("k", b)][1],
                "onescol": onescol,
            }
        )
    return in_maps


def kernel(Q_in, K_in, V_in, coords_q, coords_k, Wq, Wk, Wv, Wo, _trace=False):
    from concourse.bass_utils import run_bass_kernel_spmd

    args = [np.asarray(a, np.float32) for a in
            (Q_in, K_in, V_in, coords_q, coords_k, Wq, Wk, Wv, Wo)]
    nc = _get_program()
    in_maps = _host_prep(*args)
    res = run_bass_kernel_spmd(
        nc, in_maps, core_ids=list(range(NC_)), trace=_trace
    )
    out = np.zeros((B, L, DIM), np.float32)
    for core in range(NC_):
        out[core // 4] += res.results[core]["out_p"]
    if _trace:
        kernel._last_results = res
    return out
